# revision 12
# baseline (speedup 1.0000x reference)
"""NeuralCDE RK4 solver as a Bass/Tile kernel on 8 Trainium2 cores.

Data-parallel over batch: B=1024 -> 128 rows per core. The 127-step RK4
scan is fully unrolled. Transposed pipeline: all tensors keep batch on
the FREE dim so the per-stage recurrence never needs a PE transpose.

State z^T lives padded on 128 partitions: h -> pad(h) = (h//16)*32 + h%16
(16 live + 16 zero rows per 32-block). Per stage:
    stt  (DVE): m = fp16(z^T + alpha * k_prev^T)        [128, B]
    mm1  (PE) : h_psum[128m, B] = w1z_pad.T @ m
    relu (ACT): hS = relu(h_psum + bias1(t))  (time folded in bias)
    mm2T (PE) : fT_psum[128, 4xB] = w2_chunk.T @ hS  (4 chunks of 128
                rows each = (h_local, c) pairs; f comes out TRANSPOSED)
    tanh (ACT): fS = tanh(fT_psum)
    mul  (DVE): uT = fS * gT(step,stage)  (g pre-transposed+replicated)
    red  (PE) : k^T[pad(h), B] accumulated in PSUM via a constant 0/1
                selector stationary S32 -- the c-reduction, the
                transpose, and the RK4 combine all fall out of PSUM
                accumulation for free.
RK4 weights are pre-folded into g (k2,k3 columns hold 2x dXdt).
"""

import numpy as np

import concourse.bacc as bacc
import concourse.bass as bass
import concourse.mybir as mybir
from concourse.tile import TileContext
from concourse.bass_utils import run_bass_kernel_spmd

F32 = mybir.dt.float32
FP16 = mybir.dt.float16
B = 1024
L = 128
C_IN = 8
HID = 64
MLP_H = 128
INIT_H = 20
NSTEP = L - 1  # 127
NCORES = 8
BL = B // NCORES  # 128 batch rows per core
NF = HID * C_IN  # 512
NCH = 4  # f^T chunks of 128 rows (16 h x 8 c each)
HCH = HID // NCH  # 16 live h per chunk

_CACHE: dict = {}


def _flags():
    import os

    return (
        os.environ.get("T_RELU", "act"),      # act | dve
        int(os.environ.get("T_TANH_SPLIT", "2")),
        int(os.environ.get("T_MUL_SPLIT", "2")),
        os.environ.get("T_STT", "dve"),       # dve | pool
        int(os.environ.get("T_GDMA_SLICES", "8")),
        tuple(int(x) for x in os.environ.get("T_FILL", "3,5,4").split(",")),
    )


def _pad(h):
    return (h // HCH) * (2 * HCH) + (h % HCH)


def _build(nstep: int, with_b2: bool):
    import sys
    import time as _time

    relu_eng, tanh_split, mul_split, stt_eng, gdma_slices, fills = _flags()
    t0 = _time.time()
    nc = bacc.Bacc()
    NCLS = nstep * 3
    gt_in = nc.dram_tensor("gt", [128, NCLS * BL], FP16, kind="ExternalInput")
    b1_in = nc.dram_tensor("bias1", [MLP_H, NCLS], F32, kind="ExternalInput")
    w1z_in = nc.dram_tensor("w1z", [HID, MLP_H], FP16, kind="ExternalInput")
    w2_in = nc.dram_tensor("w2", [MLP_H, NF], FP16, kind="ExternalInput")
    s64_in = nc.dram_tensor("s64", [128, NCH * HID], FP16, kind="ExternalInput")
    b2p_in = nc.dram_tensor("b2p", [1, NF], F32, kind="ExternalInput")
    onesr_in = nc.dram_tensor("onesr", [1, BL], F32, kind="ExternalInput")
    z0t_in = nc.dram_tensor("z0t", [HID, BL], F32, kind="ExternalInput")
    m0_in = nc.dram_tensor("m0", [HID, BL], FP16, kind="ExternalInput")
    zs_out = nc.dram_tensor(
        "zs", [HID, (nstep + 1) * BL], F32, kind="ExternalOutput"
    )

    CLS = (0, 1, 1, 2)

    with TileContext(nc) as tc:
        with (
            tc.tile_pool(name="const", bufs=1) as cp,
            tc.tile_pool(name="zst", bufs=1) as zp,
            tc.tile_pool(name="ms", bufs=3) as mp,
            tc.tile_pool(name="hs", bufs=3) as hp,
            tc.tile_pool(name="fs", bufs=2) as fp,
            tc.tile_pool(name="us", bufs=3) as up,
            tc.tile_pool(name="ph", bufs=2, space="PSUM") as ph,
            tc.tile_pool(name="pf", bufs=1, space="PSUM") as pf,
            tc.tile_pool(name="pacc", bufs=1, space="PSUM") as pacc,
            tc.tile_pool(name="pks", bufs=2, space="PSUM") as pks,
            tc.tile_pool(name="pfill", bufs=1, space="PSUM") as pfl,
        ):
            gtS = cp.tile([128, NCLS * BL], FP16)
            b1S = cp.tile([MLP_H, NCLS], F32)
            w1zS = cp.tile([HID, MLP_H], FP16)
            w2S = cp.tile([MLP_H, NF], FP16)
            s64S = cp.tile([128, NCH * HID], FP16)
            b2S = cp.tile([1, NF], F32)
            onesS = cp.tile([1, BL], F32)
            m0S = cp.tile([HID, BL], FP16)
            zall = zp.tile([HID, (nstep + 1) * BL], F32)

            # gt is big (~12 MB): slice the load so step 0 isn't gated on
            # the whole transfer.
            nsl = gdma_slices
            per = (NCLS + nsl - 1) // nsl
            for i in range(nsl):
                lo = i * per * BL
                hi = min(NCLS * BL, (i + 1) * per * BL)
                if lo >= hi:
                    break
                nc.sync.dma_start(out=gtS[:, lo:hi], in_=gt_in[:, lo:hi])
            nc.sync.dma_start(out=b1S[:], in_=b1_in[:])
            nc.sync.dma_start(out=w1zS[:], in_=w1z_in[:])
            nc.sync.dma_start(out=w2S[:], in_=w2_in[:])
            nc.sync.dma_start(out=s64S[:], in_=s64_in[:])
            nc.sync.dma_start(out=b2S[:], in_=b2p_in[:])
            nc.sync.dma_start(out=onesS[:], in_=onesr_in[:])
            nc.sync.dma_start(out=m0S[:], in_=m0_in[:])
            nc.sync.dma_start(out=zall[:, 0:BL], in_=z0t_in[:])
            nc.sync.dma_start(out=zs_out[:, 0:BL], in_=z0t_in[:])

            stt = nc.vector.scalar_tensor_tensor
            fillP = pfl.tile([128, BL], F32, name="fillP")

            def fill(n, anchor):
                # matmuls that keep the PE clock from dropping to its idle
                # p-state during engine handoffs; anchored to freshly
                # produced data so the scheduler cannot hoist them
                mv = anchor[:, 0:BL]
                for _ in range(n):
                    if anchor.shape[0] == HID:
                        nc.tensor.matmul(
                            fillP[:],
                            lhsT=w1zS[:],
                            rhs=mv,
                            start=True,
                            stop=True,
                            skip_group_check=True,
                        )
                    else:
                        nc.tensor.matmul(
                            fillP[:][0:HID],
                            lhsT=s64S[:, 0:HID],
                            rhs=mv,
                            start=True,
                            stop=True,
                            skip_group_check=True,
                        )

            accP = None
            prev_ksP = None
            pending_acc = []
            for step in range(nstep):
                zT = zall[:, step * BL : (step + 1) * BL]
                for s in range(4):
                    col = step * 3 + CLS[s]
                    # ---- m (fp16 moving operand for mm1) ----
                    if step == 0 and s == 0:
                        m = m0S
                    else:
                        if s == 0:
                            ksrc, al = accP, 1.0 / 6.0
                            # z update: z_step = z_{step-1} + accP/6
                            stt(
                                out=zT,
                                in0=accP[:],
                                scalar=al,
                                in1=zall[:, (step - 1) * BL : step * BL],
                                op0=mybir.AluOpType.mult,
                                op1=mybir.AluOpType.add,
                            )
                            nc.sync.dma_start(
                                out=zs_out[:, step * BL : (step + 1) * BL],
                                in_=zT,
                            )
                            zbase = zall[:, (step - 1) * BL : step * BL]
                        elif s == 1:
                            ksrc, al = accP, 0.5
                            zbase = zT
                        else:
                            ksrc, al = prev_ksP, (0.25 if s == 2 else 0.5)
                            zbase = zT
                        m = mp.tile([HID, BL], FP16, tag="m")
                        stt(
                            out=m[:],
                            in0=ksrc[:],
                            scalar=al,
                            in1=zbase,
                            op0=mybir.AluOpType.mult,
                            op1=mybir.AluOpType.add,
                        )
                    # ---- mm1 ----
                    h_ps = ph.tile([MLP_H, BL], F32, tag="hps")
                    nc.tensor.matmul(
                        h_ps[:], lhsT=w1zS[:], rhs=m[:], start=True, stop=True
                    )
                    # flush deferred accP accumulation into the PE stream
                    # here (after mm1, before this stage's mm2T chunks)
                    flushed = len(pending_acc)
                    while pending_acc:
                        puh, paccP = pending_acc.pop(0)
                        for cch in range(NCH):
                            pu = puh[cch // (NCH // 2)]
                            rsl = slice((cch % (NCH // 2)) * BL,
                                        (cch % (NCH // 2) + 1) * BL)
                            ssl = slice(cch * HID, (cch + 1) * HID)
                            nc.tensor.matmul(
                                paccP[:],
                                lhsT=s64S[:, ssl],
                                rhs=pu[:, rsl],
                                start=False,
                                stop=True,
                                skip_group_check=True,
                            )
                    fill(max(0, fills[0] - 4 * flushed), m[:] if (step or s) else m0S[:])
                    # ---- relu (+bias with time folded in) ----
                    hS = hp.tile([MLP_H, BL], FP16, tag="hs")
                    if relu_eng == "act":
                        nc.scalar.activation(
                            hS[:],
                            h_ps[:],
                            mybir.ActivationFunctionType.Relu,
                            bias=b1S[:, col : col + 1],
                        )
                    else:
                        nc.vector.tensor_scalar(
                            hS[:],
                            h_ps[:],
                            b1S[:, col : col + 1],
                            0.0,
                            op0=mybir.AluOpType.add,
                            op1=mybir.AluOpType.max,
                        )
                    # ---- mm2 transposed (4 chunks) + tanh + mul + red ----
                    # per-half tiles: a tanh read of half 0 must not create a
                    # WAR hazard against mm2T writes of half 1
                    NH = 2
                    CPH = NCH // NH  # chunks per half
                    fTh = [pf.tile([128, CPH * BL], F32, tag=f"fps{hh}",
                                   name=f"fT{hh}") for hh in range(NH)]
                    fSh = [fp.tile([128, CPH * BL], FP16, tag=f"fs{hh}",
                                   name=f"fS{hh}") for hh in range(NH)]
                    uh = [up.tile([128, CPH * BL], FP16, tag=f"u{hh}",
                                  name=f"u{hh}") for hh in range(NH)]
                    if s == 0:
                        accP = pacc.tile([HID, BL], F32, tag="acc")
                    if s in (1, 2):
                        ksP = pks.tile([HID, BL], F32, tag="ks")
                        kdst = ksP
                    else:
                        kdst = accP
                    gcol = gtS[:, col * BL : (col + 1) * BL]
                    for hh in range(NH):
                        fT, fS, u = fTh[hh], fSh[hh], uh[hh]
                        for lc in range(CPH):
                            cch = hh * CPH + lc
                            csl = slice(lc * BL, (lc + 1) * BL)
                            if with_b2:
                                nc.tensor.matmul(
                                    fT[:, csl],
                                    lhsT=b2S[:, cch * MLP_H : (cch + 1) * MLP_H],
                                    rhs=onesS[:],
                                    start=True,
                                    stop=False,
                                )
                            nc.tensor.matmul(
                                fT[:, csl],
                                lhsT=w2S[:, cch * MLP_H : (cch + 1) * MLP_H],
                                rhs=hS[:],
                                start=not with_b2,
                                stop=True,
                            )
                        if hh == NH - 1:
                            fill(fills[1], hS[:])
                        nc.scalar.activation(
                            fS[:], fT[:], mybir.ActivationFunctionType.Tanh
                        )
                        f3 = fS[:].rearrange("p (ch b) -> p ch b", ch=CPH)
                        u3 = u[:].rearrange("p (ch b) -> p ch b", ch=CPH)
                        gvn = gcol.unsqueeze(1).broadcast_to((128, CPH, BL))
                        nc.vector.tensor_tensor(
                            out=u3, in0=f3, in1=gvn, op=mybir.AluOpType.mult
                        )
                        for lc in range(CPH):
                            cch = hh * CPH + lc
                            rsl = slice(lc * BL, (lc + 1) * BL)
                            ssl = slice(cch * HID, (cch + 1) * HID)
                            fresh = (s != 3) and cch == 0
                            nc.tensor.matmul(
                                kdst[:],
                                lhsT=s64S[:, ssl],
                                rhs=u[:, rsl],
                                start=fresh,
                                stop=(cch == NCH - 1),
                                skip_group_check=True,
                            )
                    fill(fills[2], uh[1][:])
                    if s in (1, 2):
                        # defer the accP duplicates: they are off the critical
                        # chain, so run them one stage later in PE idle time
                        pending_acc.append((uh, accP))
                        prev_ksP = ksP
            # epilogue: flush remaining accP dups, then final z
            while pending_acc:
                puh, paccP = pending_acc.pop(0)
                for cch in range(NCH):
                    pu = puh[cch // (NCH // 2)]
                    rsl = slice((cch % (NCH // 2)) * BL,
                                (cch % (NCH // 2) + 1) * BL)
                    ssl = slice(cch * HID, (cch + 1) * HID)
                    nc.tensor.matmul(
                        paccP[:],
                        lhsT=s64S[:, ssl],
                        rhs=pu[:, rsl],
                        start=False,
                        stop=True,
                        skip_group_check=True,
                    )
            zT = zall[:, nstep * BL : (nstep + 1) * BL]
            stt(
                out=zT,
                in0=accP[:],
                scalar=1.0 / 6.0,
                in1=zall[:, (nstep - 1) * BL : nstep * BL],
                op0=mybir.AluOpType.mult,
                op1=mybir.AluOpType.add,
            )
            nc.sync.dma_start(
                out=zs_out[:, nstep * BL : (nstep + 1) * BL], in_=zT
            )

    print(f"[kernel] tile trace+schedule: {_time.time()-t0:.1f}s", file=sys.stderr)
    t1 = _time.time()
    nc.finalize()
    print(f"[kernel] finalize: {_time.time()-t1:.1f}s", file=sys.stderr)
    return nc


def _get_nc(nstep: int, with_b2: bool):
    key = (nstep, with_b2) + _flags()
    if key not in _CACHE:
        _CACHE[key] = _build(nstep, with_b2)
    return _CACHE[key]


def _host_prep(coeffs, Wi1, bi1, Wi2, bi2, W1, b1, W2, b2, nstep: int):
    coeffs = np.asarray(coeffs, dtype=np.float32)
    a = coeffs[:, :, 0:8]
    b = coeffs[:, :, 8:16]
    c = coeffs[:, :, 16:24]
    d = coeffs[:, :, 24:32]

    X0 = a[:, 0]
    z0 = np.tanh(
        np.maximum(X0 @ Wi1 + bi1, 0.0).astype(np.float32) @ Wi2 + bi2
    ).astype(np.float32)

    # g with RK4 weights folded (cls1 column = 2x dXdt(t+1/2))
    g = np.empty((B, nstep, 3, C_IN), dtype=np.float32)
    g[:, :, 0] = b[:, :nstep]
    g[:, :, 1] = 2.0 * b[:, :nstep] + 2.0 * c[:, :nstep] + 1.5 * d[:, :nstep]
    last = NSTEP - 1
    for i in range(nstep):
        if i < last:
            g[:, i, 2] = b[:, i + 1]
        else:
            g[:, i, 2] = b[:, i] + 2.0 * c[:, i] + 3.0 * d[:, i]

    tcols = np.empty((nstep, 3), dtype=np.float32)
    tcols[:, 0] = np.arange(nstep, dtype=np.float32)
    tcols[:, 1] = tcols[:, 0] + 0.5
    tcols[:, 2] = tcols[:, 0] + 1.0
    bias1 = (
        b1[None, None, :] + tcols[:, :, None] * W1[0][None, None, :]
    ).astype(np.float32)
    bias1 = bias1.reshape(nstep * 3, MLP_H).T.copy()  # [128, nstep*3]

    # per-chunk selectors: s64[(hl*8+c), chunk*64 + h'] = 1 if h' == chunk*16+hl
    s64 = np.zeros((128, NCH * HID), dtype=np.float16)
    rows = np.arange(128)
    for cch in range(NCH):
        s64[rows, cch * HID + cch * HCH + rows // C_IN] = 1.0

    shared = {
        "bias1": bias1,
        "w1z": np.ascontiguousarray(W1[1:], dtype=np.float16),
        "w2": np.ascontiguousarray(W2, dtype=np.float16),
        "s64": s64,
        "b2p": np.ascontiguousarray(b2[None, :], dtype=np.float32),
        "onesr": np.ones((1, BL), dtype=np.float32),
    }
    in_maps = []
    for core in range(NCORES):
        sl = slice(core * BL, (core + 1) * BL)
        mm = dict(shared)
        # gt[r, step, cls, b] = g[b, step, cls, r % 8], replicated 16x
        gcore = g[sl].transpose(3, 1, 2, 0)  # [8, nstep, 3, BL]
        gt = np.tile(gcore, (HCH, 1, 1, 1)).reshape(128, nstep * 3 * BL)
        mm["gt"] = np.ascontiguousarray(gt.astype(np.float16))
        z0t = np.ascontiguousarray(z0[sl].T)
        mm["z0t"] = z0t
        mm["m0"] = z0t.astype(np.float16)
        in_maps.append(mm)
    return in_maps


def kernel(coeffs, Wi1, bi1, Wi2, bi2, W1, b1, W2, b2, _nstep: int = NSTEP,
           _trace: bool = False):
    import sys
    import time as _time

    nstep = _nstep
    with_b2 = bool(np.any(np.asarray(b2)))
    nc = _get_nc(nstep, with_b2)
    in_maps = _host_prep(
        coeffs, Wi1, bi1, Wi2, bi2, W1, b1, W2, b2, nstep
    )
    t0 = _time.time()
    res = run_bass_kernel_spmd(nc, in_maps, list(range(NCORES)), trace=_trace)
    print(f"[kernel] spmd run (compile+exec): {_time.time()-t0:.1f}s", file=sys.stderr)
    out = np.empty((B, nstep + 1, HID), dtype=np.float32)
    for core in range(NCORES):
        zs = res.results[core]["zs"].reshape(HID, nstep + 1, BL)
        out[core * BL : (core + 1) * BL] = zs.transpose(2, 1, 0)
    if _trace:
        kernel.last_results = res
    return out


# revision 15
# speedup vs baseline: 1.1768x; 1.1768x over previous
"""NeuralCDE RK4 solver as a Bass/Tile kernel on 8 Trainium2 cores.

Data-parallel over batch: B=1024 -> 128 rows per core; the 127-step RK4
scan is fully unrolled. Everything keeps batch on the FREE dim
(transposed pipeline), so the recurrence needs no PE transposes.

Key algebra: with u(s) = tanh(f^T(s)) * g (chunked [128=(16 h x 8 c),
B] tiles), the next stage's hidden pre-activation is
    h(s+1) = W1z^T z_base + alpha_s * W1z^T S_c^T u_c(s)
           = W1z^T z_base + (alpha_s * S_c W1z)^T u_c(s)
where S_c is the constant 0/1 c-reduction selector. Folding
W1S_c = alpha * S_c W1z (constant, fp16) lets the PE consume u(s)
DIRECTLY -- no k materialization, no DVE work on the recurrence chain.

Steady-state per-stage chain: W1S tail (2 matmuls) -> relu -> mm2T
(4 chunk matmuls) -> tanh (2 halves) -> mul by g (2 halves) -> next
stage. The z-state bookkeeping (RK4 combine via selector redmm into
PSUM, z update, output DMA) all runs off-chain in engine idle time.
RK4 weights 1,2,2,1 are pre-folded into g's stage columns.
"""

import numpy as np

import concourse.bacc as bacc
import concourse.mybir as mybir
from concourse.tile import TileContext
from concourse.bass_utils import run_bass_kernel_spmd

F32 = mybir.dt.float32
F32R = mybir.dt.float32r
FP16 = mybir.dt.float16
B = 1024
L = 128
C_IN = 8
HID = 64
MLP_H = 128
INIT_H = 20
NSTEP = L - 1  # 127
NCORES = 8
BL = B // NCORES  # 128 batch rows per core
NF = HID * C_IN  # 512
NCH = 4  # f^T chunks of 128 rows (16 h x 8 c each)
HCH = HID // NCH  # 16 live h per chunk
NH = 2  # tanh/mul halves
CPH = NCH // NH

_CACHE: dict = {}


def _flags():
    import os

    return (
        os.environ.get("T_RELU", "act"),  # act | dve
        os.environ.get("T_ZPART", "f32r"),  # f32r | f32
        int(os.environ.get("T_GDMA_SLICES", "8")),
    )


def _build(nstep: int, with_b2: bool):
    import sys
    import time as _time

    relu_eng, zpart_dt, gdma_slices = _flags()
    ZD = F32R if zpart_dt == "f32r" else F32
    t0 = _time.time()
    nc = bacc.Bacc()
    NCLS = nstep * 3
    gt_in = nc.dram_tensor("gt", [128, NCLS * BL], FP16, kind="ExternalInput")
    b1_in = nc.dram_tensor("bias1", [MLP_H, NCLS], F32, kind="ExternalInput")
    w1zr_in = nc.dram_tensor("w1zr", [HID, MLP_H], ZD, kind="ExternalInput")
    w1zh_in = nc.dram_tensor("w1zh", [HID, MLP_H], FP16, kind="ExternalInput")
    # 3 alpha sets (1/2, 1/4, 1/6) x 4 chunks of alpha * S_c @ W1z
    w1s_in = nc.dram_tensor(
        "w1s", [128, 3 * NCH * MLP_H], FP16, kind="ExternalInput"
    )
    w2_in = nc.dram_tensor("w2", [MLP_H, NF], FP16, kind="ExternalInput")
    s64_in = nc.dram_tensor("s64", [128, NCH * HID], FP16, kind="ExternalInput")
    b2p_in = nc.dram_tensor("b2p", [1, NF], F32, kind="ExternalInput")
    onesr_in = nc.dram_tensor("onesr", [1, BL], F32, kind="ExternalInput")
    z0t_in = nc.dram_tensor("z0t", [HID, BL], ZD, kind="ExternalInput")
    m0_in = nc.dram_tensor("m0", [HID, BL], FP16, kind="ExternalInput")
    zs_out = nc.dram_tensor(
        "zs", [HID, (nstep + 1) * BL], F32, kind="ExternalOutput"
    )

    CLS = (0, 1, 1, 2)
    # alpha set index per consuming stage s (who consumes u(s-1)):
    # s1 <- 1/2 (set 0), s2 <- 1/4 (set 1), s3 <- 1/2 (set 0),
    # next step's s0 <- 1/6 (set 2)
    ASET = {1: 0, 2: 1, 3: 0, 0: 2}

    with TileContext(nc) as tc:
        with (
            tc.tile_pool(name="const", bufs=1) as cp,
            tc.tile_pool(name="zst", bufs=1) as zp,
            tc.tile_pool(name="zh", bufs=2) as zhp,
            tc.tile_pool(name="hs", bufs=3) as hp,
            tc.tile_pool(name="fs", bufs=2) as fp,
            tc.tile_pool(name="us", bufs=3) as up,
            tc.tile_pool(name="ph", bufs=4, space="PSUM") as ph,
            tc.tile_pool(name="pf", bufs=1, space="PSUM") as pf,
            tc.tile_pool(name="pacc", bufs=2, space="PSUM") as pacc,
        ):
            gtS = cp.tile([128, NCLS * BL], FP16)
            b1S = cp.tile([MLP_H, NCLS], F32)
            w1zrS = cp.tile([HID, MLP_H], ZD)
            w1zhS = cp.tile([HID, MLP_H], FP16)
            w1sS = cp.tile([128, 3 * NCH * MLP_H], FP16)
            w2S = cp.tile([MLP_H, NF], FP16)
            s64S = cp.tile([128, NCH * HID], FP16)
            b2S = cp.tile([1, NF], F32)
            onesS = cp.tile([1, BL], F32)
            m0S = cp.tile([HID, BL], FP16)
            zall = zp.tile([HID, (nstep + 1) * BL], ZD)

            # gt is big (~12 MB): slice the load so step 0 isn't gated on
            # the whole transfer.
            nsl = gdma_slices
            per = (NCLS + nsl - 1) // nsl
            for i in range(nsl):
                lo = i * per * BL
                hi = min(NCLS * BL, (i + 1) * per * BL)
                if lo >= hi:
                    break
                nc.sync.dma_start(out=gtS[:, lo:hi], in_=gt_in[:, lo:hi])
            nc.sync.dma_start(out=b1S[:], in_=b1_in[:])
            nc.sync.dma_start(out=w1zrS[:], in_=w1zr_in[:])
            nc.sync.dma_start(out=w1zhS[:], in_=w1zh_in[:])
            nc.sync.dma_start(out=w1sS[:], in_=w1s_in[:])
            nc.sync.dma_start(out=w2S[:], in_=w2_in[:])
            nc.sync.dma_start(out=s64S[:], in_=s64_in[:])
            nc.sync.dma_start(out=b2S[:], in_=b2p_in[:])
            nc.sync.dma_start(out=onesS[:], in_=onesr_in[:])
            nc.sync.dma_start(out=m0S[:], in_=m0_in[:])
            nc.sync.dma_start(out=zall[:, 0:BL], in_=z0t_in[:])
            nc.sync.dma_start(out=zs_out[:, 0:BL], in_=z0t_in[:].bitcast(F32))

            stt = nc.vector.scalar_tensor_tensor
            mm = nc.tensor.matmul

            def w1s(aset, cch):
                off = (aset * NCH + cch) * MLP_H
                return w1sS[:, off : off + MLP_H]

            # one RK4 stage tail: relu -> mm2T -> tanh -> mul; returns u
            def stage_tail(h_ps, col):
                hS = hp.tile([MLP_H, BL], FP16, tag="hs", name="hS")
                if relu_eng == "act":
                    nc.scalar.activation(
                        hS[:],
                        h_ps[:],
                        mybir.ActivationFunctionType.Relu,
                        bias=b1S[:, col : col + 1],
                    )
                else:
                    nc.vector.tensor_scalar(
                        hS[:],
                        h_ps[:],
                        b1S[:, col : col + 1],
                        0.0,
                        op0=mybir.AluOpType.add,
                        op1=mybir.AluOpType.max,
                    )
                fTh = [pf.tile([128, CPH * BL], F32, tag=f"fps{hh}",
                               name=f"fT{hh}") for hh in range(NH)]
                fSh = [fp.tile([128, CPH * BL], FP16, tag=f"fs{hh}",
                               name=f"fS{hh}") for hh in range(NH)]
                uh = [up.tile([128, CPH * BL], FP16, tag=f"u{hh}",
                              name=f"u{hh}") for hh in range(NH)]
                gcol = gtS[:, col * BL : (col + 1) * BL]
                for hh in range(NH):
                    fT, fS, u = fTh[hh], fSh[hh], uh[hh]
                    for lc in range(CPH):
                        cch = hh * CPH + lc
                        csl = slice(lc * BL, (lc + 1) * BL)
                        if with_b2:
                            mm(
                                fT[:, csl],
                                lhsT=b2S[:, cch * MLP_H : (cch + 1) * MLP_H],
                                rhs=onesS[:],
                                start=True,
                                stop=False,
                            )
                        mm(
                            fT[:, csl],
                            lhsT=w2S[:, cch * MLP_H : (cch + 1) * MLP_H],
                            rhs=hS[:],
                            start=not with_b2,
                            stop=True,
                        )
                    nc.scalar.activation(
                        fS[:], fT[:], mybir.ActivationFunctionType.Tanh
                    )
                    f3 = fS[:].rearrange("p (ch b) -> p ch b", ch=CPH)
                    u3 = u[:].rearrange("p (ch b) -> p ch b", ch=CPH)
                    gvn = gcol.unsqueeze(1).broadcast_to((128, CPH, BL))
                    nc.vector.tensor_tensor(
                        out=u3, in0=f3, in1=gvn, op=mybir.AluOpType.mult
                    )
                return uh

            # W1S terms of this stage's h_ps, consuming u(s-1); one half
            def w1s_half(h_ps, uh, aset, hh, last):
                for lc in range(CPH):
                    cch = hh * CPH + lc
                    mm(
                        h_ps[:],
                        lhsT=w1s(aset, cch),
                        rhs=uh[hh][:, lc * BL : (lc + 1) * BL],
                        start=False,
                        stop=True,
                        skip_group_check=True,
                    )

            # selector redmm: u -> acc PSUM (the RK4 combine), off-chain
            def redmm(accP, uh, first):
                for cch in range(NCH):
                    mm(
                        accP[:],
                        lhsT=s64S[:, cch * HID : (cch + 1) * HID],
                        rhs=uh[cch // CPH][:, (cch % CPH) * BL
                                           : (cch % CPH + 1) * BL],
                        start=first and cch == 0,
                        stop=True,
                        skip_group_check=True,
                    )

            accP = None
            zp3h = None
            u_prev = None  # u tiles of the previous stage
            for step in range(nstep):
                zT = zall[:, step * BL : (step + 1) * BL]
                zprev = zall[:, (step - 1) * BL : step * BL]
                for s in range(4):
                    col = step * 3 + CLS[s]
                    if step == 0 and s == 0:
                        h_ps = ph.tile([MLP_H, BL], F32, tag="hps",
                                       name="h_ps")
                        mm(h_ps[:], lhsT=w1zhS[:], rhs=m0S[:], start=True,
                           stop=True)
                        u_prev = stage_tail(h_ps, col)
                        accP = pacc.tile([HID, BL], F32, tag="acc",
                                         name="accP")
                        redmm(accP, u_prev, first=True)
                        continue
                    h_ps = ph.tile([MLP_H, BL], F32, tag="hps", name="h_ps")
                    if s == 0:
                        # z-part via zpart3h = fp16(z_{i-1} + acc3/6); the
                        # missing k4/6 arrives through the W1S(1/6) terms
                        mm(h_ps[:], lhsT=w1zhS[:], rhs=zp3h[:], start=True,
                           stop=False)
                    else:
                        # z-part on the f32 base state of this step
                        mm(h_ps[:], lhsT=w1zrS[:], rhs=zT, start=True,
                           stop=False)
                    # chain tail: W1S terms consuming u(s-1)
                    aset = ASET[s]
                    w1s_half(h_ps, u_prev, aset, 0, last=False)
                    w1s_half(h_ps, u_prev, aset, 1, last=True)
                    # off-chain bookkeeping (PE slots during relu/mm2T):
                    if s == 0:
                        pend_s3 = u_prev  # u(s3 of prev step)
                    elif s == 1:
                        redmm(accP, u_prev, first=True)
                    else:
                        redmm(accP, u_prev, first=False)
                    uh = stage_tail(h_ps, col)
                    if s == 0:
                        # complete accP with u(s3), then z update + DMA
                        redmm(accP, pend_s3, first=False)
                        stt(
                            out=zT,
                            in0=accP[:],
                            scalar=1.0 / 6.0,
                            in1=zprev,
                            op0=mybir.AluOpType.mult,
                            op1=mybir.AluOpType.add,
                        )
                        nc.sync.dma_start(
                            out=zs_out[:, step * BL : (step + 1) * BL],
                            in_=zT.bitcast(F32),
                        )
                        accP = pacc.tile([HID, BL], F32, tag="acc",
                                         name="accP")
                    elif s == 3:
                        # acc3 = k1 + 2 k2 + 2 k3 complete (u_s2 folded in
                        # above); zpart3h feeds the next step's s0 z-part
                        zp3h = zhp.tile([HID, BL], FP16, tag="zp3h",
                                        name="zp3h")
                        stt(
                            out=zp3h[:],
                            in0=accP[:],
                            scalar=1.0 / 6.0,
                            in1=zT,
                            op0=mybir.AluOpType.mult,
                            op1=mybir.AluOpType.add,
                        )
                    u_prev = uh
            # epilogue: final z = z_{n-1} + accP/6 (accP needs u(s3))
            zT = zall[:, nstep * BL : (nstep + 1) * BL]
            redmm(accP, u_prev, first=False)
            stt(
                out=zT,
                in0=accP[:],
                scalar=1.0 / 6.0,
                in1=zall[:, (nstep - 1) * BL : nstep * BL],
                op0=mybir.AluOpType.mult,
                op1=mybir.AluOpType.add,
            )
            nc.sync.dma_start(
                out=zs_out[:, nstep * BL : (nstep + 1) * BL],
                in_=zT.bitcast(F32),
            )

    print(f"[kernel] tile trace+schedule: {_time.time()-t0:.1f}s", file=sys.stderr)
    t1 = _time.time()
    nc.finalize()
    print(f"[kernel] finalize: {_time.time()-t1:.1f}s", file=sys.stderr)
    return nc


def _get_nc(nstep: int, with_b2: bool):
    key = (nstep, with_b2) + _flags()
    if key not in _CACHE:
        _CACHE[key] = _build(nstep, with_b2)
    return _CACHE[key]


def _host_prep(coeffs, Wi1, bi1, Wi2, bi2, W1, b1, W2, b2, nstep: int):
    coeffs = np.asarray(coeffs, dtype=np.float32)
    a = coeffs[:, :, 0:8]
    b = coeffs[:, :, 8:16]
    c = coeffs[:, :, 16:24]
    d = coeffs[:, :, 24:32]

    X0 = a[:, 0]
    z0 = np.tanh(
        np.maximum(X0 @ Wi1 + bi1, 0.0).astype(np.float32) @ Wi2 + bi2
    ).astype(np.float32)

    # g with RK4 weights folded (cls1 column = 2x dXdt(t+1/2))
    g = np.empty((B, nstep, 3, C_IN), dtype=np.float32)
    g[:, :, 0] = b[:, :nstep]
    g[:, :, 1] = 2.0 * b[:, :nstep] + 2.0 * c[:, :nstep] + 1.5 * d[:, :nstep]
    last = NSTEP - 1
    for i in range(nstep):
        if i < last:
            g[:, i, 2] = b[:, i + 1]
        else:
            g[:, i, 2] = b[:, i] + 2.0 * c[:, i] + 3.0 * d[:, i]

    tcols = np.empty((nstep, 3), dtype=np.float32)
    tcols[:, 0] = np.arange(nstep, dtype=np.float32)
    tcols[:, 1] = tcols[:, 0] + 0.5
    tcols[:, 2] = tcols[:, 0] + 1.0
    bias1 = (
        b1[None, None, :] + tcols[:, :, None] * W1[0][None, None, :]
    ).astype(np.float32)
    bias1 = bias1.reshape(nstep * 3, MLP_H).T.copy()  # [128, nstep*3]

    # per-chunk selectors: s64[(hl*8+c), chunk*64 + h'] = 1 if h' == chunk*16+hl
    s64 = np.zeros((128, NCH * HID), dtype=np.float16)
    rows = np.arange(128)
    for cch in range(NCH):
        s64[rows, cch * HID + cch * HCH + rows // C_IN] = 1.0

    # W1S sets: alpha * (S_c @ W1z): row (hl, cin) = alpha * W1z[c*16+hl]
    w1z = W1[1:].astype(np.float32)  # [64, 128]
    w1s = np.empty((128, 3 * NCH * MLP_H), dtype=np.float16)
    for ai, alpha in enumerate((0.5, 0.25, 1.0 / 6.0)):
        for cch in range(NCH):
            blk = np.repeat(w1z[cch * HCH : (cch + 1) * HCH] * alpha,
                            C_IN, axis=0)  # [128, 128]
            w1s[:, (ai * NCH + cch) * MLP_H
                : (ai * NCH + cch + 1) * MLP_H] = blk

    zdt = np.float32  # f32r shares the f32 byte layout
    shared = {
        "bias1": bias1,
        "w1zr": np.ascontiguousarray(W1[1:], dtype=zdt),
        "w1zh": np.ascontiguousarray(W1[1:], dtype=np.float16),
        "w1s": w1s,
        "w2": np.ascontiguousarray(W2, dtype=np.float16),
        "s64": s64,
        "b2p": np.ascontiguousarray(b2[None, :], dtype=np.float32),
        "onesr": np.ones((1, BL), dtype=np.float32),
    }
    in_maps = []
    for core in range(NCORES):
        sl = slice(core * BL, (core + 1) * BL)
        mm = dict(shared)
        # gt[r, step, cls, b] = g[b, step, cls, r % 8], replicated 16x
        gcore = g[sl].transpose(3, 1, 2, 0)  # [8, nstep, 3, BL]
        gt = np.tile(gcore, (HCH, 1, 1, 1)).reshape(128, nstep * 3 * BL)
        mm["gt"] = np.ascontiguousarray(gt.astype(np.float16))
        z0t = np.ascontiguousarray(z0[sl].T)
        mm["z0t"] = z0t
        mm["m0"] = z0t.astype(np.float16)
        in_maps.append(mm)
    return in_maps


def kernel(coeffs, Wi1, bi1, Wi2, bi2, W1, b1, W2, b2, _nstep: int = NSTEP,
           _trace: bool = False):
    import sys
    import time as _time

    nstep = _nstep
    with_b2 = bool(np.any(np.asarray(b2)))
    nc = _get_nc(nstep, with_b2)
    in_maps = _host_prep(
        coeffs, Wi1, bi1, Wi2, bi2, W1, b1, W2, b2, nstep
    )
    t0 = _time.time()
    res = run_bass_kernel_spmd(nc, in_maps, list(range(NCORES)), trace=_trace)
    print(f"[kernel] spmd run (compile+exec): {_time.time()-t0:.1f}s", file=sys.stderr)
    out = np.empty((B, nstep + 1, HID), dtype=np.float32)
    for core in range(NCORES):
        zs = res.results[core]["zs"].reshape(HID, nstep + 1, BL)
        out[core * BL : (core + 1) * BL] = zs.transpose(2, 1, 0)
    if _trace:
        kernel.last_results = res
    return out


# revision 16
# speedup vs baseline: 1.1869x; 1.0086x over previous
"""NeuralCDE RK4 solver as a Bass/Tile kernel on 8 Trainium2 cores.

Data-parallel over batch: B=1024 -> 128 rows per core; the 127-step RK4
scan is fully unrolled. Everything keeps batch on the FREE dim
(transposed pipeline), so the recurrence needs no PE transposes.

Key algebra: with u(s) = tanh(f^T(s)) * g (chunked [128=(16 h x 8 c),
B] tiles), the next stage's hidden pre-activation is
    h(s+1) = W1z^T z_base + alpha_s * W1z^T S_c^T u_c(s)
           = W1z^T z_base + (alpha_s * S_c W1z)^T u_c(s)
where S_c is the constant 0/1 c-reduction selector. Folding
W1S_c = alpha * S_c W1z (constant, fp16) lets the PE consume u(s)
DIRECTLY -- no k materialization, no DVE work on the recurrence chain.

Steady-state per-stage chain: W1S tail (2 matmuls) -> relu -> mm2T
(4 chunk matmuls) -> tanh (2 halves) -> mul by g (2 halves) -> next
stage. The z-state bookkeeping (RK4 combine via selector redmm into
PSUM, z update, output DMA) all runs off-chain in engine idle time.
RK4 weights 1,2,2,1 are pre-folded into g's stage columns.
"""

import numpy as np

import concourse.bacc as bacc
import concourse.mybir as mybir
from concourse.tile import TileContext
from concourse.bass_utils import run_bass_kernel_spmd

F32 = mybir.dt.float32
F32R = mybir.dt.float32r
FP16 = mybir.dt.float16
B = 1024
L = 128
C_IN = 8
HID = 64
MLP_H = 128
INIT_H = 20
NSTEP = L - 1  # 127
NCORES = 8
BL = B // NCORES  # 128 batch rows per core
NF = HID * C_IN  # 512
NCH = 4  # f^T chunks of 128 rows (16 h x 8 c each)
HCH = HID // NCH  # 16 live h per chunk
NH = 2  # tanh/mul halves
CPH = NCH // NH

_CACHE: dict = {}


def _flags():
    import os

    return (
        os.environ.get("T_RELU", "act"),  # act | dve
        os.environ.get("T_ZPART", "f32r"),  # f32r | f32
        int(os.environ.get("T_GDMA_SLICES", "8")),
    )


def _build(nstep: int, with_b2: bool):
    import sys
    import time as _time

    relu_eng, zpart_dt, gdma_slices = _flags()
    ZD = F32R if zpart_dt == "f32r" else F32
    t0 = _time.time()
    nc = bacc.Bacc()
    NCLS = nstep * 3
    gt_in = nc.dram_tensor("gt", [128, NCLS * BL], FP16, kind="ExternalInput")
    b1_in = nc.dram_tensor("bias1", [MLP_H, NCLS], F32, kind="ExternalInput")
    w1zr_in = nc.dram_tensor("w1zr", [HID, MLP_H], ZD, kind="ExternalInput")
    w1zh_in = nc.dram_tensor("w1zh", [HID, MLP_H], FP16, kind="ExternalInput")
    # 3 alpha sets (1/2, 1/4, 1/6) x 4 chunks of alpha * S_c @ W1z
    w1s_in = nc.dram_tensor(
        "w1s", [128, 3 * NCH * MLP_H], FP16, kind="ExternalInput"
    )
    w2_in = nc.dram_tensor("w2", [MLP_H, NF], FP16, kind="ExternalInput")
    s64_in = nc.dram_tensor("s64", [128, NCH * HID], FP16, kind="ExternalInput")
    b2p_in = nc.dram_tensor("b2p", [1, NF], F32, kind="ExternalInput")
    onesr_in = nc.dram_tensor("onesr", [1, BL], F32, kind="ExternalInput")
    z0t_in = nc.dram_tensor("z0t", [HID, BL], ZD, kind="ExternalInput")
    m0_in = nc.dram_tensor("m0", [HID, BL], FP16, kind="ExternalInput")
    zs_out = nc.dram_tensor(
        "zs", [HID, (nstep + 1) * BL], F32, kind="ExternalOutput"
    )

    CLS = (0, 1, 1, 2)
    # alpha set index per consuming stage s (who consumes u(s-1)):
    # s1 <- 1/2 (set 0), s2 <- 1/4 (set 1), s3 <- 1/2 (set 0),
    # next step's s0 <- 1/6 (set 2)
    ASET = {1: 0, 2: 1, 3: 0, 0: 2}

    with TileContext(nc) as tc:
        with (
            tc.tile_pool(name="const", bufs=1) as cp,
            tc.tile_pool(name="zst", bufs=1) as zp,
            tc.tile_pool(name="zh", bufs=2) as zhp,
            tc.tile_pool(name="hs", bufs=3) as hp,
            tc.tile_pool(name="fs", bufs=2) as fp,
            tc.tile_pool(name="us", bufs=3) as up,
            tc.tile_pool(name="ph", bufs=4, space="PSUM") as ph,
            tc.tile_pool(name="pf", bufs=1, space="PSUM") as pf,
            tc.tile_pool(name="pacc", bufs=2, space="PSUM") as pacc,
        ):
            gtS = cp.tile([128, NCLS * BL], FP16)
            b1S = cp.tile([MLP_H, NCLS], F32)
            w1zrS = cp.tile([HID, MLP_H], ZD)
            w1zhS = cp.tile([HID, MLP_H], FP16)
            w1sS = cp.tile([128, 3 * NCH * MLP_H], FP16)
            w2S = cp.tile([MLP_H, NF], FP16)
            s64S = cp.tile([128, NCH * HID], FP16)
            b2S = cp.tile([1, NF], F32)
            onesS = cp.tile([1, BL], F32)
            m0S = cp.tile([HID, BL], FP16)
            zall = zp.tile([HID, (nstep + 1) * BL], ZD)

            # gt is big (~12 MB): slice the load so step 0 isn't gated on
            # the whole transfer.
            nsl = gdma_slices
            per = (NCLS + nsl - 1) // nsl
            for i in range(nsl):
                lo = i * per * BL
                hi = min(NCLS * BL, (i + 1) * per * BL)
                if lo >= hi:
                    break
                nc.sync.dma_start(out=gtS[:, lo:hi], in_=gt_in[:, lo:hi])
            nc.sync.dma_start(out=b1S[:], in_=b1_in[:])
            nc.sync.dma_start(out=w1zrS[:], in_=w1zr_in[:])
            nc.sync.dma_start(out=w1zhS[:], in_=w1zh_in[:])
            nc.sync.dma_start(out=w1sS[:], in_=w1s_in[:])
            nc.sync.dma_start(out=w2S[:], in_=w2_in[:])
            nc.sync.dma_start(out=s64S[:], in_=s64_in[:])
            nc.sync.dma_start(out=b2S[:], in_=b2p_in[:])
            nc.sync.dma_start(out=onesS[:], in_=onesr_in[:])
            nc.sync.dma_start(out=m0S[:], in_=m0_in[:])
            nc.sync.dma_start(out=zall[:, 0:BL], in_=z0t_in[:])
            nc.sync.dma_start(out=zs_out[:, 0:BL], in_=z0t_in[:].bitcast(F32))

            stt = nc.vector.scalar_tensor_tensor
            mm = nc.tensor.matmul

            def w1s(aset, cch):
                off = (aset * NCH + cch) * MLP_H
                return w1sS[:, off : off + MLP_H]

            # one RK4 stage tail: relu -> mm2T -> tanh -> mul; returns u
            def stage_tail(h_ps, col):
                hS = hp.tile([MLP_H, BL], FP16, tag="hs", name="hS")
                if relu_eng == "act":
                    nc.scalar.activation(
                        hS[:],
                        h_ps[:],
                        mybir.ActivationFunctionType.Relu,
                        bias=b1S[:, col : col + 1],
                    )
                else:
                    nc.vector.tensor_scalar(
                        hS[:],
                        h_ps[:],
                        b1S[:, col : col + 1],
                        0.0,
                        op0=mybir.AluOpType.add,
                        op1=mybir.AluOpType.max,
                    )
                fTh = [pf.tile([128, CPH * BL], F32, tag=f"fps{hh}",
                               name=f"fT{hh}") for hh in range(NH)]
                fSh = [fp.tile([128, CPH * BL], FP16, tag=f"fs{hh}",
                               name=f"fS{hh}") for hh in range(NH)]
                uh = [up.tile([128, CPH * BL], FP16, tag=f"u{hh}",
                              name=f"u{hh}") for hh in range(NH)]
                gcol = gtS[:, col * BL : (col + 1) * BL]
                for hh in range(NH):
                    fT, fS, u = fTh[hh], fSh[hh], uh[hh]
                    for lc in range(CPH):
                        cch = hh * CPH + lc
                        csl = slice(lc * BL, (lc + 1) * BL)
                        if with_b2:
                            mm(
                                fT[:, csl],
                                lhsT=b2S[:, cch * MLP_H : (cch + 1) * MLP_H],
                                rhs=onesS[:],
                                start=True,
                                stop=False,
                            )
                        mm(
                            fT[:, csl],
                            lhsT=w2S[:, cch * MLP_H : (cch + 1) * MLP_H],
                            rhs=hS[:],
                            start=not with_b2,
                            stop=True,
                        )
                    nc.scalar.activation(
                        fS[:], fT[:], mybir.ActivationFunctionType.Tanh
                    )
                    f3 = fS[:].rearrange("p (ch b) -> p ch b", ch=CPH)
                    u3 = u[:].rearrange("p (ch b) -> p ch b", ch=CPH)
                    gvn = gcol.unsqueeze(1).broadcast_to((128, CPH, BL))
                    nc.vector.tensor_tensor(
                        out=u3, in0=f3, in1=gvn, op=mybir.AluOpType.mult
                    )
                return uh

            # W1S terms of this stage's h_ps, consuming u(s-1); one half
            def w1s_half(h_ps, uh, aset, hh, last):
                for lc in range(CPH):
                    cch = hh * CPH + lc
                    mm(
                        h_ps[:],
                        lhsT=w1s(aset, cch),
                        rhs=uh[hh][:, lc * BL : (lc + 1) * BL],
                        start=False,
                        stop=True,
                        skip_group_check=True,
                    )

            # selector redmm: u -> acc PSUM (the RK4 combine), off-chain
            def redmm(accP, uh, first):
                for cch in range(NCH):
                    mm(
                        accP[:],
                        lhsT=s64S[:, cch * HID : (cch + 1) * HID],
                        rhs=uh[cch // CPH][:, (cch % CPH) * BL
                                           : (cch % CPH + 1) * BL],
                        start=first and cch == 0,
                        stop=True,
                        skip_group_check=True,
                    )

            accP = None
            zp3h = None
            u_prev = None  # u tiles of the previous stage
            for step in range(nstep):
                zT = zall[:, step * BL : (step + 1) * BL]
                zprev = zall[:, (step - 1) * BL : step * BL]
                for s in range(4):
                    col = step * 3 + CLS[s]
                    if step == 0 and s == 0:
                        zhS = m0S
                        h_ps = ph.tile([MLP_H, BL], F32, tag="hps",
                                       name="h_ps")
                        mm(h_ps[:], lhsT=w1zhS[:], rhs=m0S[:], start=True,
                           stop=True)
                        u_prev = stage_tail(h_ps, col)
                        accP = pacc.tile([HID, BL], F32, tag="acc",
                                         name="accP")
                        redmm(accP, u_prev, first=True)
                        continue
                    h_ps = ph.tile([MLP_H, BL], F32, tag="hps", name="h_ps")
                    if s == 0:
                        # z-part via zpart3h = fp16(z_{i-1} + acc3/6); the
                        # missing k4/6 arrives through the W1S(1/6) terms
                        mm(h_ps[:], lhsT=w1zhS[:], rhs=zp3h[:], start=True,
                           stop=False)
                    else:
                        # z-part on the fp16 copy of this step's base state
                        mm(h_ps[:], lhsT=w1zhS[:], rhs=zhS[:], start=True,
                           stop=False)
                    # chain tail: W1S terms consuming u(s-1)
                    aset = ASET[s]
                    w1s_half(h_ps, u_prev, aset, 0, last=False)
                    w1s_half(h_ps, u_prev, aset, 1, last=True)
                    if s == 0:
                        pend_s3 = u_prev  # u(s3 of prev step)
                    uh = stage_tail(h_ps, col)
                    # off-chain bookkeeping: selector redmm of u(s-1),
                    # emitted after the stage tail so the scheduler
                    # backfills it into the tanh/mul window
                    if s == 1:
                        redmm(accP, u_prev, first=True)
                    elif s in (2, 3):
                        redmm(accP, u_prev, first=False)
                    if s == 0:
                        # complete accP with u(s3), then z update + DMA
                        redmm(accP, pend_s3, first=False)
                        stt(
                            out=zT,
                            in0=accP[:],
                            scalar=1.0 / 6.0,
                            in1=zprev,
                            op0=mybir.AluOpType.mult,
                            op1=mybir.AluOpType.add,
                        )
                        nc.sync.dma_start(
                            out=zs_out[:, step * BL : (step + 1) * BL],
                            in_=zT.bitcast(F32),
                        )
                        zhS = zhp.tile([HID, BL], FP16, tag="zh2",
                                       name="zhS")
                        stt(
                            out=zhS[:],
                            in0=accP[:],
                            scalar=1.0 / 6.0,
                            in1=zprev,
                            op0=mybir.AluOpType.mult,
                            op1=mybir.AluOpType.add,
                        )
                        accP = pacc.tile([HID, BL], F32, tag="acc",
                                         name="accP")
                    elif s == 3:
                        # acc3 = k1 + 2 k2 + 2 k3 complete (u_s2 folded in
                        # above); zpart3h feeds the next step's s0 z-part
                        zp3h = zhp.tile([HID, BL], FP16, tag="zp3h",
                                        name="zp3h")
                        stt(
                            out=zp3h[:],
                            in0=accP[:],
                            scalar=1.0 / 6.0,
                            in1=zT,
                            op0=mybir.AluOpType.mult,
                            op1=mybir.AluOpType.add,
                        )
                    u_prev = uh
            # epilogue: final z = z_{n-1} + accP/6 (accP needs u(s3))
            zT = zall[:, nstep * BL : (nstep + 1) * BL]
            redmm(accP, u_prev, first=False)
            stt(
                out=zT,
                in0=accP[:],
                scalar=1.0 / 6.0,
                in1=zall[:, (nstep - 1) * BL : nstep * BL],
                op0=mybir.AluOpType.mult,
                op1=mybir.AluOpType.add,
            )
            nc.sync.dma_start(
                out=zs_out[:, nstep * BL : (nstep + 1) * BL],
                in_=zT.bitcast(F32),
            )

    print(f"[kernel] tile trace+schedule: {_time.time()-t0:.1f}s", file=sys.stderr)
    t1 = _time.time()
    nc.finalize()
    print(f"[kernel] finalize: {_time.time()-t1:.1f}s", file=sys.stderr)
    return nc


def _get_nc(nstep: int, with_b2: bool):
    key = (nstep, with_b2) + _flags()
    if key not in _CACHE:
        _CACHE[key] = _build(nstep, with_b2)
    return _CACHE[key]


def _host_prep(coeffs, Wi1, bi1, Wi2, bi2, W1, b1, W2, b2, nstep: int):
    coeffs = np.asarray(coeffs, dtype=np.float32)
    a = coeffs[:, :, 0:8]
    b = coeffs[:, :, 8:16]
    c = coeffs[:, :, 16:24]
    d = coeffs[:, :, 24:32]

    X0 = a[:, 0]
    z0 = np.tanh(
        np.maximum(X0 @ Wi1 + bi1, 0.0).astype(np.float32) @ Wi2 + bi2
    ).astype(np.float32)

    # g with RK4 weights folded (cls1 column = 2x dXdt(t+1/2))
    g = np.empty((B, nstep, 3, C_IN), dtype=np.float32)
    g[:, :, 0] = b[:, :nstep]
    g[:, :, 1] = 2.0 * b[:, :nstep] + 2.0 * c[:, :nstep] + 1.5 * d[:, :nstep]
    last = NSTEP - 1
    for i in range(nstep):
        if i < last:
            g[:, i, 2] = b[:, i + 1]
        else:
            g[:, i, 2] = b[:, i] + 2.0 * c[:, i] + 3.0 * d[:, i]

    tcols = np.empty((nstep, 3), dtype=np.float32)
    tcols[:, 0] = np.arange(nstep, dtype=np.float32)
    tcols[:, 1] = tcols[:, 0] + 0.5
    tcols[:, 2] = tcols[:, 0] + 1.0
    bias1 = (
        b1[None, None, :] + tcols[:, :, None] * W1[0][None, None, :]
    ).astype(np.float32)
    bias1 = bias1.reshape(nstep * 3, MLP_H).T.copy()  # [128, nstep*3]

    # per-chunk selectors: s64[(hl*8+c), chunk*64 + h'] = 1 if h' == chunk*16+hl
    s64 = np.zeros((128, NCH * HID), dtype=np.float16)
    rows = np.arange(128)
    for cch in range(NCH):
        s64[rows, cch * HID + cch * HCH + rows // C_IN] = 1.0

    # W1S sets: alpha * (S_c @ W1z): row (hl, cin) = alpha * W1z[c*16+hl]
    w1z = W1[1:].astype(np.float32)  # [64, 128]
    w1s = np.empty((128, 3 * NCH * MLP_H), dtype=np.float16)
    for ai, alpha in enumerate((0.5, 0.25, 1.0 / 6.0)):
        for cch in range(NCH):
            blk = np.repeat(w1z[cch * HCH : (cch + 1) * HCH] * alpha,
                            C_IN, axis=0)  # [128, 128]
            w1s[:, (ai * NCH + cch) * MLP_H
                : (ai * NCH + cch + 1) * MLP_H] = blk

    zdt = np.float32  # f32r shares the f32 byte layout
    shared = {
        "bias1": bias1,
        "w1zr": np.ascontiguousarray(W1[1:], dtype=zdt),
        "w1zh": np.ascontiguousarray(W1[1:], dtype=np.float16),
        "w1s": w1s,
        "w2": np.ascontiguousarray(W2, dtype=np.float16),
        "s64": s64,
        "b2p": np.ascontiguousarray(b2[None, :], dtype=np.float32),
        "onesr": np.ones((1, BL), dtype=np.float32),
    }
    in_maps = []
    for core in range(NCORES):
        sl = slice(core * BL, (core + 1) * BL)
        mm = dict(shared)
        # gt[r, step, cls, b] = g[b, step, cls, r % 8], replicated 16x
        gcore = g[sl].transpose(3, 1, 2, 0)  # [8, nstep, 3, BL]
        gt = np.tile(gcore, (HCH, 1, 1, 1)).reshape(128, nstep * 3 * BL)
        mm["gt"] = np.ascontiguousarray(gt.astype(np.float16))
        z0t = np.ascontiguousarray(z0[sl].T)
        mm["z0t"] = z0t
        mm["m0"] = z0t.astype(np.float16)
        in_maps.append(mm)
    return in_maps


def kernel(coeffs, Wi1, bi1, Wi2, bi2, W1, b1, W2, b2, _nstep: int = NSTEP,
           _trace: bool = False):
    import sys
    import time as _time

    nstep = _nstep
    with_b2 = bool(np.any(np.asarray(b2)))
    nc = _get_nc(nstep, with_b2)
    in_maps = _host_prep(
        coeffs, Wi1, bi1, Wi2, bi2, W1, b1, W2, b2, nstep
    )
    t0 = _time.time()
    res = run_bass_kernel_spmd(nc, in_maps, list(range(NCORES)), trace=_trace)
    print(f"[kernel] spmd run (compile+exec): {_time.time()-t0:.1f}s", file=sys.stderr)
    out = np.empty((B, nstep + 1, HID), dtype=np.float32)
    for core in range(NCORES):
        zs = res.results[core]["zs"].reshape(HID, nstep + 1, BL)
        out[core * BL : (core + 1) * BL] = zs.transpose(2, 1, 0)
    if _trace:
        kernel.last_results = res
    return out


# revision 17
# speedup vs baseline: 1.2017x; 1.0124x over previous
"""NeuralCDE RK4 solver as a Bass/Tile kernel on 8 Trainium2 cores.

Data-parallel over batch: B=1024 -> 128 rows per core; the 127-step RK4
scan is fully unrolled. Everything keeps batch on the FREE dim
(transposed pipeline), so the recurrence needs no PE transposes.

Key algebra: with u(s) = tanh(f^T(s)) * g (chunked [128=(16 h x 8 c),
B] tiles), the next stage's hidden pre-activation is
    h(s+1) = W1z^T z_base + alpha_s * W1z^T S_c^T u_c(s)
           = W1z^T z_base + (alpha_s * S_c W1z)^T u_c(s)
where S_c is the constant 0/1 c-reduction selector. Folding
W1S_c = alpha * S_c W1z (constant, fp16) lets the PE consume u(s)
DIRECTLY -- no k materialization, no DVE work on the recurrence chain.

Steady-state per-stage chain: W1S tail (2 matmuls) -> relu -> mm2T
(4 chunk matmuls) -> tanh (2 halves) -> mul by g (2 halves) -> next
stage. The z-state bookkeeping (RK4 combine via selector redmm into
PSUM, z update, output DMA) all runs off-chain in engine idle time.
RK4 weights 1,2,2,1 are pre-folded into g's stage columns.
"""

import numpy as np

import concourse.bacc as bacc
import concourse.mybir as mybir
from concourse.tile import TileContext
from concourse.bass_utils import run_bass_kernel_spmd

F32 = mybir.dt.float32
F32R = mybir.dt.float32r
FP16 = mybir.dt.float16
B = 1024
L = 128
C_IN = 8
HID = 64
MLP_H = 128
INIT_H = 20
NSTEP = L - 1  # 127
NCORES = 8
BL = B // NCORES  # 128 batch rows per core
NF = HID * C_IN  # 512
NCH = 4  # f^T chunks of 128 rows (16 h x 8 c each)
HCH = HID // NCH  # 16 live h per chunk
NH = 2  # tanh/mul halves
CPH = NCH // NH

_CACHE: dict = {}


def _flags():
    import os

    return (
        os.environ.get("T_RELU", "act"),  # act | dve
        os.environ.get("T_ZPART", "f32r"),  # f32r | f32
        int(os.environ.get("T_GDMA_SLICES", "8")),
    )


def _build(nstep: int, with_b2: bool):
    import sys
    import time as _time

    relu_eng, zpart_dt, gdma_slices = _flags()
    ZD = F32R if zpart_dt == "f32r" else F32
    t0 = _time.time()
    nc = bacc.Bacc()
    NCLS = nstep * 3
    gt_in = nc.dram_tensor("gt", [128, NCLS * BL], FP16, kind="ExternalInput")
    b1_in = nc.dram_tensor("bias1", [MLP_H, NCLS], F32, kind="ExternalInput")
    w1zr_in = nc.dram_tensor("w1zr", [HID, MLP_H], ZD, kind="ExternalInput")
    w1zh_in = nc.dram_tensor("w1zh", [HID, MLP_H], FP16, kind="ExternalInput")
    # 3 alpha sets (1/2, 1/4, 1/6) x 4 chunks of alpha * S_c @ W1z
    w1s_in = nc.dram_tensor(
        "w1s", [128, 3 * NCH * MLP_H], FP16, kind="ExternalInput"
    )
    w2_in = nc.dram_tensor("w2", [MLP_H, NF], FP16, kind="ExternalInput")
    s64_in = nc.dram_tensor("s64", [128, NCH * HID], FP16, kind="ExternalInput")
    b2p_in = nc.dram_tensor("b2p", [1, NF], F32, kind="ExternalInput")
    onesr_in = nc.dram_tensor("onesr", [1, BL], F32, kind="ExternalInput")
    z0t_in = nc.dram_tensor("z0t", [HID, BL], ZD, kind="ExternalInput")
    m0_in = nc.dram_tensor("m0", [HID, BL], FP16, kind="ExternalInput")
    zs_out = nc.dram_tensor(
        "zs", [HID, (nstep + 1) * BL], F32, kind="ExternalOutput"
    )

    CLS = (0, 1, 1, 2)
    # alpha set index per consuming stage s (who consumes u(s-1)):
    # s1 <- 1/2 (set 0), s2 <- 1/4 (set 1), s3 <- 1/2 (set 0),
    # next step's s0 <- 1/6 (set 2)
    ASET = {1: 0, 2: 1, 3: 0, 0: 2}

    with TileContext(nc) as tc:
        with (
            tc.tile_pool(name="const", bufs=1) as cp,
            tc.tile_pool(name="zst", bufs=1) as zp,
            tc.tile_pool(name="zh", bufs=2) as zhp,
            tc.tile_pool(name="hs", bufs=3) as hp,
            tc.tile_pool(name="fs", bufs=2) as fp,
            tc.tile_pool(name="us", bufs=3) as up,
            tc.tile_pool(name="usum", bufs=2) as usp,
            tc.tile_pool(name="ph", bufs=4, space="PSUM") as ph,
            tc.tile_pool(name="pf", bufs=1, space="PSUM") as pf,
            tc.tile_pool(name="pacc", bufs=2, space="PSUM") as pacc,
        ):
            gtS = cp.tile([128, NCLS * BL], FP16)
            b1S = cp.tile([MLP_H, NCLS], F32)
            w1zrS = cp.tile([HID, MLP_H], ZD)
            w1zhS = cp.tile([HID, MLP_H], FP16)
            w1sS = cp.tile([128, 3 * NCH * MLP_H], FP16)
            w2S = cp.tile([MLP_H, NF], FP16)
            s64S = cp.tile([128, NCH * HID], FP16)
            b2S = cp.tile([1, NF], F32)
            onesS = cp.tile([1, BL], F32)
            m0S = cp.tile([HID, BL], FP16)
            zall = zp.tile([HID, (nstep + 1) * BL], ZD)

            # gt is big (~12 MB): slice the load so step 0 isn't gated on
            # the whole transfer.
            nsl = gdma_slices
            per = (NCLS + nsl - 1) // nsl
            for i in range(nsl):
                lo = i * per * BL
                hi = min(NCLS * BL, (i + 1) * per * BL)
                if lo >= hi:
                    break
                nc.sync.dma_start(out=gtS[:, lo:hi], in_=gt_in[:, lo:hi])
            nc.sync.dma_start(out=b1S[:], in_=b1_in[:])
            nc.sync.dma_start(out=w1zrS[:], in_=w1zr_in[:])
            nc.sync.dma_start(out=w1zhS[:], in_=w1zh_in[:])
            nc.sync.dma_start(out=w1sS[:], in_=w1s_in[:])
            nc.sync.dma_start(out=w2S[:], in_=w2_in[:])
            nc.sync.dma_start(out=s64S[:], in_=s64_in[:])
            nc.sync.dma_start(out=b2S[:], in_=b2p_in[:])
            nc.sync.dma_start(out=onesS[:], in_=onesr_in[:])
            nc.sync.dma_start(out=m0S[:], in_=m0_in[:])
            nc.sync.dma_start(out=zall[:, 0:BL], in_=z0t_in[:])
            nc.sync.dma_start(out=zs_out[:, 0:BL], in_=z0t_in[:].bitcast(F32))

            stt = nc.vector.scalar_tensor_tensor
            mm = nc.tensor.matmul

            def w1s(aset, cch):
                off = (aset * NCH + cch) * MLP_H
                return w1sS[:, off : off + MLP_H]

            # one RK4 stage tail: relu -> mm2T -> tanh -> mul; returns u
            def stage_tail(h_ps, col):
                hS = hp.tile([MLP_H, BL], FP16, tag="hs", name="hS")
                if relu_eng == "act":
                    nc.scalar.activation(
                        hS[:],
                        h_ps[:],
                        mybir.ActivationFunctionType.Relu,
                        bias=b1S[:, col : col + 1],
                    )
                else:
                    nc.vector.tensor_scalar(
                        hS[:],
                        h_ps[:],
                        b1S[:, col : col + 1],
                        0.0,
                        op0=mybir.AluOpType.add,
                        op1=mybir.AluOpType.max,
                    )
                fTh = [pf.tile([128, CPH * BL], F32, tag=f"fps{hh}",
                               name=f"fT{hh}") for hh in range(NH)]
                fSh = [fp.tile([128, CPH * BL], FP16, tag=f"fs{hh}",
                               name=f"fS{hh}") for hh in range(NH)]
                uh = [up.tile([128, CPH * BL], FP16, tag=f"u{hh}",
                              name=f"u{hh}") for hh in range(NH)]
                gcol = gtS[:, col * BL : (col + 1) * BL]
                for hh in range(NH):
                    fT, fS, u = fTh[hh], fSh[hh], uh[hh]
                    for lc in range(CPH):
                        cch = hh * CPH + lc
                        csl = slice(lc * BL, (lc + 1) * BL)
                        if with_b2:
                            mm(
                                fT[:, csl],
                                lhsT=b2S[:, cch * MLP_H : (cch + 1) * MLP_H],
                                rhs=onesS[:],
                                start=True,
                                stop=False,
                            )
                        mm(
                            fT[:, csl],
                            lhsT=w2S[:, cch * MLP_H : (cch + 1) * MLP_H],
                            rhs=hS[:],
                            start=not with_b2,
                            stop=True,
                        )
                    nc.scalar.activation(
                        fS[:], fT[:], mybir.ActivationFunctionType.Tanh
                    )
                    f3 = fS[:].rearrange("p (ch b) -> p ch b", ch=CPH)
                    u3 = u[:].rearrange("p (ch b) -> p ch b", ch=CPH)
                    gvn = gcol.unsqueeze(1).broadcast_to((128, CPH, BL))
                    nc.vector.tensor_tensor(
                        out=u3, in0=f3, in1=gvn, op=mybir.AluOpType.mult
                    )
                return uh

            # W1S terms of this stage's h_ps, consuming u(s-1); one half
            def w1s_half(h_ps, uh, aset, hh, last):
                for lc in range(CPH):
                    cch = hh * CPH + lc
                    mm(
                        h_ps[:],
                        lhsT=w1s(aset, cch),
                        rhs=uh[hh][:, lc * BL : (lc + 1) * BL],
                        start=False,
                        stop=True,
                        skip_group_check=True,
                    )

            # selector redmm: u -> acc PSUM (the RK4 combine), off-chain
            def redmm(accP, uh, first):
                for cch in range(NCH):
                    mm(
                        accP[:],
                        lhsT=s64S[:, cch * HID : (cch + 1) * HID],
                        rhs=uh[cch // CPH][:, (cch % CPH) * BL
                                           : (cch % CPH + 1) * BL],
                        start=first and cch == 0,
                        stop=True,
                        skip_group_check=True,
                    )

            accP = None
            zp3h = None
            u_prev = None  # u tiles of the previous stage
            for step in range(nstep):
                zT = zall[:, step * BL : (step + 1) * BL]
                zprev = zall[:, (step - 1) * BL : step * BL]
                for s in range(4):
                    col = step * 3 + CLS[s]
                    if step == 0 and s == 0:
                        zhS = m0S
                        h_ps = ph.tile([MLP_H, BL], F32, tag="hps",
                                       name="h_ps")
                        mm(h_ps[:], lhsT=w1zhS[:], rhs=m0S[:], start=True,
                           stop=True)
                        u_prev = stage_tail(h_ps, col)
                        accP = pacc.tile([HID, BL], F32, tag="acc",
                                         name="accP")
                        redmm(accP, u_prev, first=True)
                        continue
                    h_ps = ph.tile([MLP_H, BL], F32, tag="hps", name="h_ps")
                    if s == 0:
                        # z-part via zpart3h = fp16(z_{i-1} + acc3/6); the
                        # missing k4/6 arrives through the W1S(1/6) terms
                        mm(h_ps[:], lhsT=w1zhS[:], rhs=zp3h[:], start=True,
                           stop=False)
                    else:
                        # z-part on the fp16 copy of this step's base state
                        mm(h_ps[:], lhsT=w1zhS[:], rhs=zhS[:], start=True,
                           stop=False)
                    # chain tail: W1S terms consuming u(s-1)
                    aset = ASET[s]
                    w1s_half(h_ps, u_prev, aset, 0, last=False)
                    w1s_half(h_ps, u_prev, aset, 1, last=True)
                    if s == 0:
                        pend_s3 = u_prev  # u(s3 of prev step)
                    uh = stage_tail(h_ps, col)
                    # off-chain: accumulate u(s-1) into the step's u-sum on
                    # the (idle) Vector engine; the RK4 combine then needs
                    # only two selector-redmm groups per step
                    if s == 1:
                        usum = [usp.tile([128, CPH * BL], FP16,
                                         tag=f"usum{hh}", name=f"usum{hh}")
                                for hh in range(NH)]
                        for hh in range(NH):
                            nc.vector.tensor_tensor(
                                out=usum[hh][:], in0=u_prev[hh][:],
                                in1=uh[hh][:], op=mybir.AluOpType.add,
                            )
                    elif s == 2:
                        for hh in range(NH):
                            nc.vector.tensor_tensor(
                                out=usum[hh][:], in0=usum[hh][:],
                                in1=uh[hh][:], op=mybir.AluOpType.add,
                            )
                    elif s == 3:
                        # acc3 = k1 + 2k2 + 2k3 via one redmm group
                        redmm(accP, usum, first=True)
                    if s == 0:
                        # complete accP with u(s3), then z update + DMA
                        redmm(accP, pend_s3, first=False)
                        stt(
                            out=zT,
                            in0=accP[:],
                            scalar=1.0 / 6.0,
                            in1=zprev,
                            op0=mybir.AluOpType.mult,
                            op1=mybir.AluOpType.add,
                        )
                        nc.sync.dma_start(
                            out=zs_out[:, step * BL : (step + 1) * BL],
                            in_=zT.bitcast(F32),
                        )
                        zhS = zhp.tile([HID, BL], FP16, tag="zh2",
                                       name="zhS")
                        stt(
                            out=zhS[:],
                            in0=accP[:],
                            scalar=1.0 / 6.0,
                            in1=zprev,
                            op0=mybir.AluOpType.mult,
                            op1=mybir.AluOpType.add,
                        )
                        accP = pacc.tile([HID, BL], F32, tag="acc",
                                         name="accP")
                    elif s == 3:
                        # acc3 = k1 + 2 k2 + 2 k3 complete (u_s2 folded in
                        # above); zpart3h feeds the next step's s0 z-part
                        zp3h = zhp.tile([HID, BL], FP16, tag="zp3h",
                                        name="zp3h")
                        stt(
                            out=zp3h[:],
                            in0=accP[:],
                            scalar=1.0 / 6.0,
                            in1=zT,
                            op0=mybir.AluOpType.mult,
                            op1=mybir.AluOpType.add,
                        )
                    u_prev = uh
            # epilogue: final z = z_{n-1} + accP/6 (accP needs u(s3))
            zT = zall[:, nstep * BL : (nstep + 1) * BL]
            redmm(accP, u_prev, first=False)
            stt(
                out=zT,
                in0=accP[:],
                scalar=1.0 / 6.0,
                in1=zall[:, (nstep - 1) * BL : nstep * BL],
                op0=mybir.AluOpType.mult,
                op1=mybir.AluOpType.add,
            )
            nc.sync.dma_start(
                out=zs_out[:, nstep * BL : (nstep + 1) * BL],
                in_=zT.bitcast(F32),
            )

    print(f"[kernel] tile trace+schedule: {_time.time()-t0:.1f}s", file=sys.stderr)
    t1 = _time.time()
    nc.finalize()
    print(f"[kernel] finalize: {_time.time()-t1:.1f}s", file=sys.stderr)
    return nc


def _get_nc(nstep: int, with_b2: bool):
    key = (nstep, with_b2) + _flags()
    if key not in _CACHE:
        _CACHE[key] = _build(nstep, with_b2)
    return _CACHE[key]


def _host_prep(coeffs, Wi1, bi1, Wi2, bi2, W1, b1, W2, b2, nstep: int):
    coeffs = np.asarray(coeffs, dtype=np.float32)
    a = coeffs[:, :, 0:8]
    b = coeffs[:, :, 8:16]
    c = coeffs[:, :, 16:24]
    d = coeffs[:, :, 24:32]

    X0 = a[:, 0]
    z0 = np.tanh(
        np.maximum(X0 @ Wi1 + bi1, 0.0).astype(np.float32) @ Wi2 + bi2
    ).astype(np.float32)

    # g with RK4 weights folded (cls1 column = 2x dXdt(t+1/2))
    g = np.empty((B, nstep, 3, C_IN), dtype=np.float32)
    g[:, :, 0] = b[:, :nstep]
    g[:, :, 1] = 2.0 * b[:, :nstep] + 2.0 * c[:, :nstep] + 1.5 * d[:, :nstep]
    last = NSTEP - 1
    for i in range(nstep):
        if i < last:
            g[:, i, 2] = b[:, i + 1]
        else:
            g[:, i, 2] = b[:, i] + 2.0 * c[:, i] + 3.0 * d[:, i]

    tcols = np.empty((nstep, 3), dtype=np.float32)
    tcols[:, 0] = np.arange(nstep, dtype=np.float32)
    tcols[:, 1] = tcols[:, 0] + 0.5
    tcols[:, 2] = tcols[:, 0] + 1.0
    bias1 = (
        b1[None, None, :] + tcols[:, :, None] * W1[0][None, None, :]
    ).astype(np.float32)
    bias1 = bias1.reshape(nstep * 3, MLP_H).T.copy()  # [128, nstep*3]

    # per-chunk selectors: s64[(hl*8+c), chunk*64 + h'] = 1 if h' == chunk*16+hl
    s64 = np.zeros((128, NCH * HID), dtype=np.float16)
    rows = np.arange(128)
    for cch in range(NCH):
        s64[rows, cch * HID + cch * HCH + rows // C_IN] = 1.0

    # W1S sets: alpha * (S_c @ W1z): row (hl, cin) = alpha * W1z[c*16+hl]
    w1z = W1[1:].astype(np.float32)  # [64, 128]
    w1s = np.empty((128, 3 * NCH * MLP_H), dtype=np.float16)
    for ai, alpha in enumerate((0.5, 0.25, 1.0 / 6.0)):
        for cch in range(NCH):
            blk = np.repeat(w1z[cch * HCH : (cch + 1) * HCH] * alpha,
                            C_IN, axis=0)  # [128, 128]
            w1s[:, (ai * NCH + cch) * MLP_H
                : (ai * NCH + cch + 1) * MLP_H] = blk

    zdt = np.float32  # f32r shares the f32 byte layout
    shared = {
        "bias1": bias1,
        "w1zr": np.ascontiguousarray(W1[1:], dtype=zdt),
        "w1zh": np.ascontiguousarray(W1[1:], dtype=np.float16),
        "w1s": w1s,
        "w2": np.ascontiguousarray(W2, dtype=np.float16),
        "s64": s64,
        "b2p": np.ascontiguousarray(b2[None, :], dtype=np.float32),
        "onesr": np.ones((1, BL), dtype=np.float32),
    }
    in_maps = []
    for core in range(NCORES):
        sl = slice(core * BL, (core + 1) * BL)
        mm = dict(shared)
        # gt[r, step, cls, b] = g[b, step, cls, r % 8], replicated 16x
        gcore = g[sl].transpose(3, 1, 2, 0)  # [8, nstep, 3, BL]
        gt = np.tile(gcore, (HCH, 1, 1, 1)).reshape(128, nstep * 3 * BL)
        mm["gt"] = np.ascontiguousarray(gt.astype(np.float16))
        z0t = np.ascontiguousarray(z0[sl].T)
        mm["z0t"] = z0t
        mm["m0"] = z0t.astype(np.float16)
        in_maps.append(mm)
    return in_maps


def kernel(coeffs, Wi1, bi1, Wi2, bi2, W1, b1, W2, b2, _nstep: int = NSTEP,
           _trace: bool = False):
    import sys
    import time as _time

    nstep = _nstep
    with_b2 = bool(np.any(np.asarray(b2)))
    nc = _get_nc(nstep, with_b2)
    in_maps = _host_prep(
        coeffs, Wi1, bi1, Wi2, bi2, W1, b1, W2, b2, nstep
    )
    t0 = _time.time()
    res = run_bass_kernel_spmd(nc, in_maps, list(range(NCORES)), trace=_trace)
    print(f"[kernel] spmd run (compile+exec): {_time.time()-t0:.1f}s", file=sys.stderr)
    out = np.empty((B, nstep + 1, HID), dtype=np.float32)
    for core in range(NCORES):
        zs = res.results[core]["zs"].reshape(HID, nstep + 1, BL)
        out[core * BL : (core + 1) * BL] = zs.transpose(2, 1, 0)
    if _trace:
        kernel.last_results = res
    return out


# revision 19
# speedup vs baseline: 1.2533x; 1.0429x over previous
"""NeuralCDE RK4 solver as a Bass/Tile kernel on 8 Trainium2 cores.

Data-parallel over batch: B=1024 -> 128 rows per core; the 127-step RK4
scan is fully unrolled. Everything keeps batch on the FREE dim
(transposed pipeline), so the recurrence needs no PE transposes.

Key algebra: with u(s) = tanh(f^T(s)) * g (chunked [128=(16 h x 8 c),
B] tiles), the next stage's hidden pre-activation is
    h(s+1) = W1z^T z_base + alpha_s * W1z^T S_c^T u_c(s)
           = W1z^T z_base + (alpha_s * S_c W1z)^T u_c(s)
where S_c is the constant 0/1 c-reduction selector. Folding
W1S_c = alpha * S_c W1z (constant, fp16) lets the PE consume u(s)
DIRECTLY -- no k materialization, no DVE work on the recurrence chain.

Steady-state per-stage chain: W1S tail (2 matmuls) -> relu -> mm2T
(4 chunk matmuls) -> tanh (2 halves) -> mul by g (2 halves) -> next
stage. The z-state bookkeeping (RK4 combine via selector redmm into
PSUM, z update, output DMA) all runs off-chain in engine idle time.
RK4 weights 1,2,2,1 are pre-folded into g's stage columns.
"""

import numpy as np

import concourse.bacc as bacc
import concourse.mybir as mybir
from concourse.tile import TileContext
from concourse.bass_utils import run_bass_kernel_spmd

F32 = mybir.dt.float32
F32R = mybir.dt.float32r
FP16 = mybir.dt.float16
B = 1024
L = 128
C_IN = 8
HID = 64
MLP_H = 128
INIT_H = 20
NSTEP = L - 1  # 127
NCORES = 8
BL = B // NCORES  # 128 batch rows per core
NF = HID * C_IN  # 512
NCH = 4  # f^T chunks of 128 rows (16 h x 8 c each)
HCH = HID // NCH  # 16 live h per chunk
NH = 2  # tanh/mul halves
CPH = NCH // NH

_CACHE: dict = {}


def _flags():
    import os

    return (
        os.environ.get("T_RELU", "act"),  # act | dve
        os.environ.get("T_ZPART", "f32r"),  # f32r | f32
        int(os.environ.get("T_GDMA_SLICES", "8")),
    )


def _build(nstep: int, with_b2: bool):
    import sys
    import time as _time

    relu_eng, zpart_dt, gdma_slices = _flags()
    ZD = F32R if zpart_dt == "f32r" else F32
    t0 = _time.time()
    nc = bacc.Bacc()
    NCLS = nstep * 3
    gt_in = nc.dram_tensor("gt", [128, NCLS * BL], FP16, kind="ExternalInput")
    b1_in = nc.dram_tensor("bias1", [MLP_H, NCLS], F32, kind="ExternalInput")
    w1zr_in = nc.dram_tensor("w1zr", [HID, MLP_H], ZD, kind="ExternalInput")
    w1zh_in = nc.dram_tensor("w1zh", [HID, MLP_H], FP16, kind="ExternalInput")
    # 3 alpha sets (1/2, 1/4, 1/6) x 4 chunks of alpha * S_c @ W1z
    w1s_in = nc.dram_tensor(
        "w1s", [128, 3 * NCH * MLP_H], FP16, kind="ExternalInput"
    )
    w2_in = nc.dram_tensor("w2", [MLP_H, NF], FP16, kind="ExternalInput")
    s64_in = nc.dram_tensor("s64", [128, NCH * HID], FP16, kind="ExternalInput")
    b2p_in = nc.dram_tensor("b2p", [1, NF], F32, kind="ExternalInput")
    onesr_in = nc.dram_tensor("onesr", [1, BL], F32, kind="ExternalInput")
    z0t_in = nc.dram_tensor("z0t", [HID, BL], ZD, kind="ExternalInput")
    m0_in = nc.dram_tensor("m0", [HID, BL], FP16, kind="ExternalInput")
    zs_out = nc.dram_tensor(
        "zs", [HID, (nstep + 1) * BL], F32, kind="ExternalOutput"
    )

    CLS = (0, 1, 1, 2)
    # alpha set index per consuming stage s (who consumes u(s-1)):
    # s1 <- 1/2 (set 0), s2 <- 1/4 (set 1), s3 <- 1/2 (set 0),
    # next step's s0 <- 1/6 (set 2)
    ASET = {1: 0, 2: 1, 3: 0, 0: 2}

    with TileContext(nc) as tc:
        with (
            tc.tile_pool(name="const", bufs=1) as cp,
            tc.tile_pool(name="zst", bufs=1) as zp,
            tc.tile_pool(name="zh", bufs=2) as zhp,
            tc.tile_pool(name="hs", bufs=3) as hp,
            tc.tile_pool(name="fs", bufs=2) as fp,
            tc.tile_pool(name="us", bufs=3) as up,
            tc.tile_pool(name="usum", bufs=2) as usp,
            tc.tile_pool(name="uall", bufs=2) as uap,
            tc.tile_pool(name="ph", bufs=4, space="PSUM") as ph,
            tc.tile_pool(name="pf", bufs=1, space="PSUM") as pf,
            tc.tile_pool(name="pacc", bufs=2, space="PSUM") as pacc,
        ):
            gtS = cp.tile([128, NCLS * BL], FP16)
            b1S = cp.tile([MLP_H, NCLS], F32)
            w1zrS = cp.tile([HID, MLP_H], ZD)
            w1zhS = cp.tile([HID, MLP_H], FP16)
            w1sS = cp.tile([128, 3 * NCH * MLP_H], FP16)
            w2S = cp.tile([MLP_H, NF], FP16)
            s64S = cp.tile([128, NCH * HID], FP16)
            b2S = cp.tile([1, NF], F32)
            onesS = cp.tile([1, BL], F32)
            m0S = cp.tile([HID, BL], FP16)
            zall = zp.tile([HID, (nstep + 1) * BL], ZD)

            # gt is big (~12 MB): slice the load so step 0 isn't gated on
            # the whole transfer.
            nsl = gdma_slices
            per = (NCLS + nsl - 1) // nsl
            for i in range(nsl):
                lo = i * per * BL
                hi = min(NCLS * BL, (i + 1) * per * BL)
                if lo >= hi:
                    break
                nc.sync.dma_start(out=gtS[:, lo:hi], in_=gt_in[:, lo:hi])
            nc.sync.dma_start(out=b1S[:], in_=b1_in[:])
            nc.sync.dma_start(out=w1zrS[:], in_=w1zr_in[:])
            nc.sync.dma_start(out=w1zhS[:], in_=w1zh_in[:])
            nc.sync.dma_start(out=w1sS[:], in_=w1s_in[:])
            nc.sync.dma_start(out=w2S[:], in_=w2_in[:])
            nc.sync.dma_start(out=s64S[:], in_=s64_in[:])
            nc.sync.dma_start(out=b2S[:], in_=b2p_in[:])
            nc.sync.dma_start(out=onesS[:], in_=onesr_in[:])
            nc.sync.dma_start(out=m0S[:], in_=m0_in[:])
            nc.sync.dma_start(out=zall[:, 0:BL], in_=z0t_in[:])
            nc.sync.dma_start(out=zs_out[:, 0:BL], in_=z0t_in[:].bitcast(F32))

            stt = nc.vector.scalar_tensor_tensor
            mm = nc.tensor.matmul

            def w1s(aset, cch):
                off = (aset * NCH + cch) * MLP_H
                return w1sS[:, off : off + MLP_H]

            # one RK4 stage tail: relu -> mm2T -> tanh -> mul; returns u
            def stage_tail(h_ps, col):
                hS = hp.tile([MLP_H, BL], FP16, tag="hs", name="hS")
                if relu_eng == "act":
                    nc.scalar.activation(
                        hS[:],
                        h_ps[:],
                        mybir.ActivationFunctionType.Relu,
                        bias=b1S[:, col : col + 1],
                    )
                else:
                    nc.vector.tensor_scalar(
                        hS[:],
                        h_ps[:],
                        b1S[:, col : col + 1],
                        0.0,
                        op0=mybir.AluOpType.add,
                        op1=mybir.AluOpType.max,
                    )
                fTh = [pf.tile([128, CPH * BL], F32, tag=f"fps{hh}",
                               name=f"fT{hh}") for hh in range(NH)]
                fSh = [fp.tile([128, CPH * BL], FP16, tag=f"fs{hh}",
                               name=f"fS{hh}") for hh in range(NH)]
                uh = [up.tile([128, CPH * BL], FP16, tag=f"u{hh}",
                              name=f"u{hh}") for hh in range(NH)]
                gcol = gtS[:, col * BL : (col + 1) * BL]
                for hh in range(NH):
                    fT, fS, u = fTh[hh], fSh[hh], uh[hh]
                    for lc in range(CPH):
                        cch = hh * CPH + lc
                        csl = slice(lc * BL, (lc + 1) * BL)
                        if with_b2:
                            mm(
                                fT[:, csl],
                                lhsT=b2S[:, cch * MLP_H : (cch + 1) * MLP_H],
                                rhs=onesS[:],
                                start=True,
                                stop=False,
                            )
                        mm(
                            fT[:, csl],
                            lhsT=w2S[:, cch * MLP_H : (cch + 1) * MLP_H],
                            rhs=hS[:],
                            start=not with_b2,
                            stop=True,
                        )
                    nc.scalar.activation(
                        fS[:], fT[:], mybir.ActivationFunctionType.Tanh
                    )
                    f3 = fS[:].rearrange("p (ch b) -> p ch b", ch=CPH)
                    u3 = u[:].rearrange("p (ch b) -> p ch b", ch=CPH)
                    gvn = gcol.unsqueeze(1).broadcast_to((128, CPH, BL))
                    nc.vector.tensor_tensor(
                        out=u3, in0=f3, in1=gvn, op=mybir.AluOpType.mult
                    )
                return uh

            # W1S terms of this stage's h_ps, consuming u(s-1); one half
            def w1s_half(h_ps, uh, aset, hh, last):
                for lc in range(CPH):
                    cch = hh * CPH + lc
                    mm(
                        h_ps[:],
                        lhsT=w1s(aset, cch),
                        rhs=uh[hh][:, lc * BL : (lc + 1) * BL],
                        start=False,
                        stop=True,
                        skip_group_check=True,
                    )

            # selector redmm: u -> acc PSUM (the RK4 combine), off-chain
            def redmm(accP, uh, first):
                for cch in range(NCH):
                    mm(
                        accP[:],
                        lhsT=s64S[:, cch * HID : (cch + 1) * HID],
                        rhs=uh[cch // CPH][:, (cch % CPH) * BL
                                           : (cch % CPH + 1) * BL],
                        start=first and cch == 0,
                        stop=True,
                        skip_group_check=True,
                    )

            accP = None
            zhS = None
            u_prev = None  # u tiles of the previous stage
            usum = None
            usum_prev = None
            for step in range(nstep):
                zT = zall[:, step * BL : (step + 1) * BL]
                zprev = zall[:, (step - 1) * BL : step * BL]
                for s in range(4):
                    col = step * 3 + CLS[s]
                    if step == 0 and s == 0:
                        zhS = m0S
                        h_ps = ph.tile([MLP_H, BL], F32, tag="hps",
                                       name="h_ps")
                        mm(h_ps[:], lhsT=w1zhS[:], rhs=m0S[:], start=True,
                           stop=True)
                        u_prev = stage_tail(h_ps, col)
                        continue
                    h_ps = ph.tile([MLP_H, BL], F32, tag="hps", name="h_ps")
                    aset = ASET[s]
                    if s == 0:
                        # h = W1z^T zh_{i-1} + (1/6) W1S (usum_prev + u_s3):
                        # the full z_i never enters the chain
                        mm(h_ps[:], lhsT=w1zhS[:], rhs=zhS[:],
                           start=True, stop=False)
                        for cch in range(NCH):
                            mm(
                                h_ps[:],
                                lhsT=w1s(aset, cch),
                                rhs=usum_prev[cch // CPH][
                                    :, (cch % CPH) * BL : (cch % CPH + 1) * BL
                                ],
                                start=False,
                                stop=True,
                                skip_group_check=True,
                            )
                    else:
                        mm(h_ps[:], lhsT=w1zhS[:], rhs=zhS[:], start=True,
                           stop=False)
                    # chain tail: W1S terms consuming u(s-1)
                    w1s_half(h_ps, u_prev, aset, 0, last=False)
                    w1s_half(h_ps, u_prev, aset, 1, last=True)
                    if s == 0:
                        pend_s3 = u_prev  # u(s3 of prev step)
                    uh = stage_tail(h_ps, col)
                    # off-chain bookkeeping on the (mostly idle) Vector
                    # engine: u-sums make every selector-redmm input ready
                    # early, so the PE backfills it into the tanh window
                    if s == 1:
                        usum = [usp.tile([128, CPH * BL], FP16,
                                         tag=f"usum{hh}", name=f"usum{hh}")
                                for hh in range(NH)]
                        for hh in range(NH):
                            nc.vector.tensor_tensor(
                                out=usum[hh][:], in0=u_prev[hh][:],
                                in1=uh[hh][:], op=mybir.AluOpType.add,
                            )
                    elif s == 2:
                        for hh in range(NH):
                            nc.vector.tensor_tensor(
                                out=usum[hh][:], in0=usum[hh][:],
                                in1=uh[hh][:], op=mybir.AluOpType.add,
                            )
                    if s == 0:
                        # uall = usum_prev + u_s3; accP = redmm(uall);
                        # z_i = z_{i-1} + accP/6; zh_i = fp16(z_i)
                        uall = [uap.tile([128, CPH * BL], FP16,
                                         tag=f"uall{hh}", name=f"uall{hh}")
                                for hh in range(NH)]
                        for hh in range(NH):
                            nc.vector.tensor_tensor(
                                out=uall[hh][:], in0=usum_prev[hh][:],
                                in1=pend_s3[hh][:], op=mybir.AluOpType.add,
                            )
                        accP = pacc.tile([HID, BL], F32, tag="acc",
                                         name="accP")
                        redmm(accP, uall, first=True)
                        stt(
                            out=zT,
                            in0=accP[:],
                            scalar=1.0 / 6.0,
                            in1=zprev,
                            op0=mybir.AluOpType.mult,
                            op1=mybir.AluOpType.add,
                        )
                        nc.sync.dma_start(
                            out=zs_out[:, step * BL : (step + 1) * BL],
                            in_=zT.bitcast(F32),
                        )
                        zhS = zhp.tile([HID, BL], FP16, tag="zh2",
                                       name="zhS")
                        stt(
                            out=zhS[:],
                            in0=accP[:],
                            scalar=1.0 / 6.0,
                            in1=zprev,
                            op0=mybir.AluOpType.mult,
                            op1=mybir.AluOpType.add,
                        )
                    elif s == 3:
                        usum_prev = usum
                    u_prev = uh
            # epilogue: final z = z_{n-1} + accP/6
            zT = zall[:, nstep * BL : (nstep + 1) * BL]
            uallE = [uap.tile([128, CPH * BL], FP16, tag=f"uall{hh}",
                              name=f"uallE{hh}") for hh in range(NH)]
            for hh in range(NH):
                nc.vector.tensor_tensor(
                    out=uallE[hh][:], in0=usum_prev[hh][:],
                    in1=u_prev[hh][:], op=mybir.AluOpType.add,
                )
            accP = pacc.tile([HID, BL], F32, tag="acc", name="accPE")
            redmm(accP, uallE, first=True)
            stt(
                out=zT,
                in0=accP[:],
                scalar=1.0 / 6.0,
                in1=zall[:, (nstep - 1) * BL : nstep * BL],
                op0=mybir.AluOpType.mult,
                op1=mybir.AluOpType.add,
            )
            nc.sync.dma_start(
                out=zs_out[:, nstep * BL : (nstep + 1) * BL],
                in_=zT.bitcast(F32),
            )

    print(f"[kernel] tile trace+schedule: {_time.time()-t0:.1f}s", file=sys.stderr)
    t1 = _time.time()
    nc.finalize()
    print(f"[kernel] finalize: {_time.time()-t1:.1f}s", file=sys.stderr)
    return nc


def _get_nc(nstep: int, with_b2: bool):
    key = (nstep, with_b2) + _flags()
    if key not in _CACHE:
        _CACHE[key] = _build(nstep, with_b2)
    return _CACHE[key]


def _host_prep(coeffs, Wi1, bi1, Wi2, bi2, W1, b1, W2, b2, nstep: int):
    coeffs = np.asarray(coeffs, dtype=np.float32)
    a = coeffs[:, :, 0:8]
    b = coeffs[:, :, 8:16]
    c = coeffs[:, :, 16:24]
    d = coeffs[:, :, 24:32]

    X0 = a[:, 0]
    z0 = np.tanh(
        np.maximum(X0 @ Wi1 + bi1, 0.0).astype(np.float32) @ Wi2 + bi2
    ).astype(np.float32)

    # g with RK4 weights folded (cls1 column = 2x dXdt(t+1/2))
    g = np.empty((B, nstep, 3, C_IN), dtype=np.float32)
    g[:, :, 0] = b[:, :nstep]
    g[:, :, 1] = 2.0 * b[:, :nstep] + 2.0 * c[:, :nstep] + 1.5 * d[:, :nstep]
    last = NSTEP - 1
    for i in range(nstep):
        if i < last:
            g[:, i, 2] = b[:, i + 1]
        else:
            g[:, i, 2] = b[:, i] + 2.0 * c[:, i] + 3.0 * d[:, i]

    tcols = np.empty((nstep, 3), dtype=np.float32)
    tcols[:, 0] = np.arange(nstep, dtype=np.float32)
    tcols[:, 1] = tcols[:, 0] + 0.5
    tcols[:, 2] = tcols[:, 0] + 1.0
    bias1 = (
        b1[None, None, :] + tcols[:, :, None] * W1[0][None, None, :]
    ).astype(np.float32)
    bias1 = bias1.reshape(nstep * 3, MLP_H).T.copy()  # [128, nstep*3]

    # per-chunk selectors: s64[(hl*8+c), chunk*64 + h'] = 1 if h' == chunk*16+hl
    s64 = np.zeros((128, NCH * HID), dtype=np.float16)
    rows = np.arange(128)
    for cch in range(NCH):
        s64[rows, cch * HID + cch * HCH + rows // C_IN] = 1.0

    # W1S sets: alpha * (S_c @ W1z): row (hl, cin) = alpha * W1z[c*16+hl]
    w1z = W1[1:].astype(np.float32)  # [64, 128]
    w1s = np.empty((128, 3 * NCH * MLP_H), dtype=np.float16)
    for ai, alpha in enumerate((0.5, 0.25, 1.0 / 6.0)):
        for cch in range(NCH):
            blk = np.repeat(w1z[cch * HCH : (cch + 1) * HCH] * alpha,
                            C_IN, axis=0)  # [128, 128]
            w1s[:, (ai * NCH + cch) * MLP_H
                : (ai * NCH + cch + 1) * MLP_H] = blk

    zdt = np.float32  # f32r shares the f32 byte layout
    shared = {
        "bias1": bias1,
        "w1zr": np.ascontiguousarray(W1[1:], dtype=zdt),
        "w1zh": np.ascontiguousarray(W1[1:], dtype=np.float16),
        "w1s": w1s,
        "w2": np.ascontiguousarray(W2, dtype=np.float16),
        "s64": s64,
        "b2p": np.ascontiguousarray(b2[None, :], dtype=np.float32),
        "onesr": np.ones((1, BL), dtype=np.float32),
    }
    in_maps = []
    for core in range(NCORES):
        sl = slice(core * BL, (core + 1) * BL)
        mm = dict(shared)
        # gt[r, step, cls, b] = g[b, step, cls, r % 8], replicated 16x
        gcore = g[sl].transpose(3, 1, 2, 0)  # [8, nstep, 3, BL]
        gt = np.tile(gcore, (HCH, 1, 1, 1)).reshape(128, nstep * 3 * BL)
        mm["gt"] = np.ascontiguousarray(gt.astype(np.float16))
        z0t = np.ascontiguousarray(z0[sl].T)
        mm["z0t"] = z0t
        mm["m0"] = z0t.astype(np.float16)
        in_maps.append(mm)
    return in_maps


def kernel(coeffs, Wi1, bi1, Wi2, bi2, W1, b1, W2, b2, _nstep: int = NSTEP,
           _trace: bool = False):
    import sys
    import time as _time

    nstep = _nstep
    with_b2 = bool(np.any(np.asarray(b2)))
    nc = _get_nc(nstep, with_b2)
    in_maps = _host_prep(
        coeffs, Wi1, bi1, Wi2, bi2, W1, b1, W2, b2, nstep
    )
    t0 = _time.time()
    res = run_bass_kernel_spmd(nc, in_maps, list(range(NCORES)), trace=_trace)
    print(f"[kernel] spmd run (compile+exec): {_time.time()-t0:.1f}s", file=sys.stderr)
    out = np.empty((B, nstep + 1, HID), dtype=np.float32)
    for core in range(NCORES):
        zs = res.results[core]["zs"].reshape(HID, nstep + 1, BL)
        out[core * BL : (core + 1) * BL] = zs.transpose(2, 1, 0)
    if _trace:
        kernel.last_results = res
    return out


# revision 20
# speedup vs baseline: 1.2774x; 1.0193x over previous
"""NeuralCDE RK4 solver as a Bass/Tile kernel on 8 Trainium2 cores.

Data-parallel over batch: B=1024 -> 128 rows per core; the 127-step RK4
scan is fully unrolled. Everything keeps batch on the FREE dim
(transposed pipeline), so the recurrence needs no PE transposes.

Key algebra: with u(s) = tanh(f^T(s)) * g (chunked [128=(16 h x 8 c),
B] tiles), the next stage's hidden pre-activation is
    h(s+1) = W1z^T z_base + alpha_s * W1z^T S_c^T u_c(s)
           = W1z^T z_base + (alpha_s * S_c W1z)^T u_c(s)
where S_c is the constant 0/1 c-reduction selector. Folding
W1S_c = alpha * S_c W1z (constant, fp16) lets the PE consume u(s)
DIRECTLY -- no k materialization, no DVE work on the recurrence chain.

Steady-state per-stage chain: W1S tail (2 matmuls) -> relu -> mm2T
(4 chunk matmuls) -> tanh (2 halves) -> mul by g (2 halves) -> next
stage. The z-state bookkeeping (RK4 combine via selector redmm into
PSUM, z update, output DMA) all runs off-chain in engine idle time.
RK4 weights 1,2,2,1 are pre-folded into g's stage columns.
"""

import numpy as np

import concourse.bacc as bacc
import concourse.mybir as mybir
from concourse.tile import TileContext
from concourse.bass_utils import run_bass_kernel_spmd

F32 = mybir.dt.float32
F32R = mybir.dt.float32r
FP16 = mybir.dt.float16
B = 1024
L = 128
C_IN = 8
HID = 64
MLP_H = 128
INIT_H = 20
NSTEP = L - 1  # 127
NCORES = 8
BL = B // NCORES  # 128 batch rows per core
NF = HID * C_IN  # 512
NCH = 4  # f^T chunks of 128 rows (16 h x 8 c each)
HCH = HID // NCH  # 16 live h per chunk
NH = 2  # tanh/mul halves
CPH = NCH // NH

_CACHE: dict = {}


def _flags():
    import os

    return (
        os.environ.get("T_RELU", "act"),  # act | dve
        os.environ.get("T_USUM", "pool"),  # pool | dve
        os.environ.get("T_ZPART", "f32r"),  # f32r | f32
        int(os.environ.get("T_GDMA_SLICES", "32")),
    )


def _build(nstep: int, with_b2: bool):
    import sys
    import time as _time

    relu_eng, usum_eng, zpart_dt, gdma_slices = _flags()
    ZD = F32R if zpart_dt == "f32r" else F32
    t0 = _time.time()
    nc = bacc.Bacc()
    NCLS = nstep * 3
    gt_in = nc.dram_tensor("gt", [128, NCLS * BL], FP16, kind="ExternalInput")
    b1_in = nc.dram_tensor("bias1", [MLP_H, NCLS], F32, kind="ExternalInput")
    w1zr_in = nc.dram_tensor("w1zr", [HID, MLP_H], ZD, kind="ExternalInput")
    w1zh_in = nc.dram_tensor("w1zh", [HID, MLP_H], FP16, kind="ExternalInput")
    # 3 alpha sets (1/2, 1/4, 1/6) x 4 chunks of alpha * S_c @ W1z
    w1s_in = nc.dram_tensor(
        "w1s", [128, 3 * NCH * MLP_H], FP16, kind="ExternalInput"
    )
    w2_in = nc.dram_tensor("w2", [MLP_H, NF], FP16, kind="ExternalInput")
    s64_in = nc.dram_tensor("s64", [128, NCH * HID], FP16, kind="ExternalInput")
    b2p_in = nc.dram_tensor("b2p", [1, NF], F32, kind="ExternalInput")
    onesr_in = nc.dram_tensor("onesr", [1, BL], F32, kind="ExternalInput")
    z0t_in = nc.dram_tensor("z0t", [HID, BL], ZD, kind="ExternalInput")
    m0_in = nc.dram_tensor("m0", [HID, BL], FP16, kind="ExternalInput")
    zs_out = nc.dram_tensor(
        "zs", [HID, (nstep + 1) * BL], F32, kind="ExternalOutput"
    )

    CLS = (0, 1, 1, 2)
    # alpha set index per consuming stage s (who consumes u(s-1)):
    # s1 <- 1/2 (set 0), s2 <- 1/4 (set 1), s3 <- 1/2 (set 0),
    # next step's s0 <- 1/6 (set 2)
    ASET = {1: 0, 2: 1, 3: 0, 0: 2}

    with TileContext(nc) as tc:
        with (
            tc.tile_pool(name="const", bufs=1) as cp,
            tc.tile_pool(name="zst", bufs=1) as zp,
            tc.tile_pool(name="zh", bufs=2) as zhp,
            tc.tile_pool(name="hs", bufs=3) as hp,
            tc.tile_pool(name="fs", bufs=2) as fp,
            tc.tile_pool(name="us", bufs=3) as up,
            tc.tile_pool(name="usum", bufs=2) as usp,
            tc.tile_pool(name="uall", bufs=2) as uap,
            tc.tile_pool(name="ph", bufs=4, space="PSUM") as ph,
            tc.tile_pool(name="pf", bufs=1, space="PSUM") as pf,
            tc.tile_pool(name="pacc", bufs=2, space="PSUM") as pacc,
        ):
            gtS = cp.tile([128, NCLS * BL], FP16)
            b1S = cp.tile([MLP_H, NCLS], F32)
            w1zrS = cp.tile([HID, MLP_H], ZD)
            w1zhS = cp.tile([HID, MLP_H], FP16)
            w1sS = cp.tile([128, 3 * NCH * MLP_H], FP16)
            w2S = cp.tile([MLP_H, NF], FP16)
            s64S = cp.tile([128, NCH * HID], FP16)
            b2S = cp.tile([1, NF], F32)
            onesS = cp.tile([1, BL], F32)
            m0S = cp.tile([HID, BL], FP16)
            zall = zp.tile([HID, (nstep + 1) * BL], ZD)

            nc.sync.dma_start(out=b1S[:], in_=b1_in[:])
            nc.sync.dma_start(out=w1zrS[:], in_=w1zr_in[:])
            nc.sync.dma_start(out=w1zhS[:], in_=w1zh_in[:])
            nc.sync.dma_start(out=w1sS[:], in_=w1s_in[:])
            nc.sync.dma_start(out=w2S[:], in_=w2_in[:])
            nc.sync.dma_start(out=s64S[:], in_=s64_in[:])
            nc.sync.dma_start(out=b2S[:], in_=b2p_in[:])
            nc.sync.dma_start(out=onesS[:], in_=onesr_in[:])
            nc.sync.dma_start(out=m0S[:], in_=m0_in[:])
            nc.sync.dma_start(out=zall[:, 0:BL], in_=z0t_in[:])
            nc.sync.dma_start(out=zs_out[:, 0:BL], in_=z0t_in[:].bitcast(F32))
            # gt is big (~12 MB): load it in slices, after the small consts,
            # so step 0 isn't gated on the whole transfer
            nsl = gdma_slices
            per = (NCLS + nsl - 1) // nsl
            for i in range(nsl):
                lo = i * per * BL
                hi = min(NCLS * BL, (i + 1) * per * BL)
                if lo >= hi:
                    break
                nc.sync.dma_start(out=gtS[:, lo:hi], in_=gt_in[:, lo:hi])

            stt = nc.vector.scalar_tensor_tensor
            mm = nc.tensor.matmul
            uadd_tt = (nc.gpsimd.tensor_tensor if usum_eng == "pool"
                       else nc.vector.tensor_tensor)

            def w1s(aset, cch):
                off = (aset * NCH + cch) * MLP_H
                return w1sS[:, off : off + MLP_H]

            # one RK4 stage tail: relu -> mm2T -> tanh -> mul; returns u
            def stage_tail(h_ps, col):
                hS = hp.tile([MLP_H, BL], FP16, tag="hs", name="hS")
                if relu_eng == "act":
                    nc.scalar.activation(
                        hS[:],
                        h_ps[:],
                        mybir.ActivationFunctionType.Relu,
                        bias=b1S[:, col : col + 1],
                    )
                else:
                    nc.vector.tensor_scalar(
                        hS[:],
                        h_ps[:],
                        b1S[:, col : col + 1],
                        0.0,
                        op0=mybir.AluOpType.add,
                        op1=mybir.AluOpType.max,
                    )
                fTh = [pf.tile([128, CPH * BL], F32, tag=f"fps{hh}",
                               name=f"fT{hh}") for hh in range(NH)]
                fSh = [fp.tile([128, CPH * BL], FP16, tag=f"fs{hh}",
                               name=f"fS{hh}") for hh in range(NH)]
                uh = [up.tile([128, CPH * BL], FP16, tag=f"u{hh}",
                              name=f"u{hh}") for hh in range(NH)]
                gcol = gtS[:, col * BL : (col + 1) * BL]
                for hh in range(NH):
                    fT, fS, u = fTh[hh], fSh[hh], uh[hh]
                    for lc in range(CPH):
                        cch = hh * CPH + lc
                        csl = slice(lc * BL, (lc + 1) * BL)
                        if with_b2:
                            mm(
                                fT[:, csl],
                                lhsT=b2S[:, cch * MLP_H : (cch + 1) * MLP_H],
                                rhs=onesS[:],
                                start=True,
                                stop=False,
                            )
                        mm(
                            fT[:, csl],
                            lhsT=w2S[:, cch * MLP_H : (cch + 1) * MLP_H],
                            rhs=hS[:],
                            start=not with_b2,
                            stop=True,
                        )
                    nc.scalar.activation(
                        fS[:], fT[:], mybir.ActivationFunctionType.Tanh
                    )
                    f3 = fS[:].rearrange("p (ch b) -> p ch b", ch=CPH)
                    u3 = u[:].rearrange("p (ch b) -> p ch b", ch=CPH)
                    gvn = gcol.unsqueeze(1).broadcast_to((128, CPH, BL))
                    nc.vector.tensor_tensor(
                        out=u3, in0=f3, in1=gvn, op=mybir.AluOpType.mult
                    )
                return uh

            # W1S terms of this stage's h_ps, consuming u(s-1); one half
            def w1s_half(h_ps, uh, aset, hh, last):
                for lc in range(CPH):
                    cch = hh * CPH + lc
                    mm(
                        h_ps[:],
                        lhsT=w1s(aset, cch),
                        rhs=uh[hh][:, lc * BL : (lc + 1) * BL],
                        start=False,
                        stop=True,
                        skip_group_check=True,
                    )

            # selector redmm: u -> acc PSUM (the RK4 combine), off-chain
            def redmm(accP, uh, first):
                for cch in range(NCH):
                    mm(
                        accP[:],
                        lhsT=s64S[:, cch * HID : (cch + 1) * HID],
                        rhs=uh[cch // CPH][:, (cch % CPH) * BL
                                           : (cch % CPH + 1) * BL],
                        start=first and cch == 0,
                        stop=True,
                        skip_group_check=True,
                    )

            accP = None
            zhS = None
            u_prev = None  # u tiles of the previous stage
            usum = None
            usum_prev = None
            for step in range(nstep):
                zT = zall[:, step * BL : (step + 1) * BL]
                zprev = zall[:, (step - 1) * BL : step * BL]
                for s in range(4):
                    col = step * 3 + CLS[s]
                    if step == 0 and s == 0:
                        zhS = m0S
                        h_ps = ph.tile([MLP_H, BL], F32, tag="hps",
                                       name="h_ps")
                        mm(h_ps[:], lhsT=w1zhS[:], rhs=m0S[:], start=True,
                           stop=True)
                        u_prev = stage_tail(h_ps, col)
                        continue
                    h_ps = ph.tile([MLP_H, BL], F32, tag="hps", name="h_ps")
                    aset = ASET[s]
                    if s == 0:
                        # h = W1z^T zh_{i-1} + (1/6) W1S (usum_prev + u_s3):
                        # the full z_i never enters the chain
                        mm(h_ps[:], lhsT=w1zhS[:], rhs=zhS[:],
                           start=True, stop=False)
                        for cch in range(NCH):
                            mm(
                                h_ps[:],
                                lhsT=w1s(aset, cch),
                                rhs=usum_prev[cch // CPH][
                                    :, (cch % CPH) * BL : (cch % CPH + 1) * BL
                                ],
                                start=False,
                                stop=True,
                                skip_group_check=True,
                            )
                    else:
                        mm(h_ps[:], lhsT=w1zhS[:], rhs=zhS[:], start=True,
                           stop=False)
                    # chain tail: W1S terms consuming u(s-1)
                    w1s_half(h_ps, u_prev, aset, 0, last=False)
                    w1s_half(h_ps, u_prev, aset, 1, last=True)
                    if s == 0:
                        pend_s3 = u_prev  # u(s3 of prev step)
                    uh = stage_tail(h_ps, col)
                    # off-chain bookkeeping on the (mostly idle) Vector
                    # engine: u-sums make every selector-redmm input ready
                    # early, so the PE backfills it into the tanh window
                    if s == 1:
                        usum = [usp.tile([128, CPH * BL], FP16,
                                         tag=f"usum{hh}", name=f"usum{hh}")
                                for hh in range(NH)]
                        for hh in range(NH):
                            uadd_tt(
                                out=usum[hh][:], in0=u_prev[hh][:],
                                in1=uh[hh][:], op=mybir.AluOpType.add,
                            )
                    elif s == 2:
                        for hh in range(NH):
                            uadd_tt(
                                out=usum[hh][:], in0=usum[hh][:],
                                in1=uh[hh][:], op=mybir.AluOpType.add,
                            )
                    if s == 0:
                        # uall = usum_prev + u_s3; accP = redmm(uall);
                        # z_i = z_{i-1} + accP/6; zh_i = fp16(z_i)
                        uall = [uap.tile([128, CPH * BL], FP16,
                                         tag=f"uall{hh}", name=f"uall{hh}")
                                for hh in range(NH)]
                        for hh in range(NH):
                            uadd_tt(
                                out=uall[hh][:], in0=usum_prev[hh][:],
                                in1=pend_s3[hh][:], op=mybir.AluOpType.add,
                            )
                        accP = pacc.tile([HID, BL], F32, tag="acc",
                                         name="accP")
                        redmm(accP, uall, first=True)
                        stt(
                            out=zT,
                            in0=accP[:],
                            scalar=1.0 / 6.0,
                            in1=zprev,
                            op0=mybir.AluOpType.mult,
                            op1=mybir.AluOpType.add,
                        )
                        nc.sync.dma_start(
                            out=zs_out[:, step * BL : (step + 1) * BL],
                            in_=zT.bitcast(F32),
                        )
                        zhS = zhp.tile([HID, BL], FP16, tag="zh2",
                                       name="zhS")
                        stt(
                            out=zhS[:],
                            in0=accP[:],
                            scalar=1.0 / 6.0,
                            in1=zprev,
                            op0=mybir.AluOpType.mult,
                            op1=mybir.AluOpType.add,
                        )
                    elif s == 3:
                        usum_prev = usum
                    u_prev = uh
            # epilogue: final z = z_{n-1} + accP/6
            zT = zall[:, nstep * BL : (nstep + 1) * BL]
            uallE = [uap.tile([128, CPH * BL], FP16, tag=f"uall{hh}",
                              name=f"uallE{hh}") for hh in range(NH)]
            for hh in range(NH):
                uadd_tt(
                    out=uallE[hh][:], in0=usum_prev[hh][:],
                    in1=u_prev[hh][:], op=mybir.AluOpType.add,
                )
            accP = pacc.tile([HID, BL], F32, tag="acc", name="accPE")
            redmm(accP, uallE, first=True)
            stt(
                out=zT,
                in0=accP[:],
                scalar=1.0 / 6.0,
                in1=zall[:, (nstep - 1) * BL : nstep * BL],
                op0=mybir.AluOpType.mult,
                op1=mybir.AluOpType.add,
            )
            nc.sync.dma_start(
                out=zs_out[:, nstep * BL : (nstep + 1) * BL],
                in_=zT.bitcast(F32),
            )

    print(f"[kernel] tile trace+schedule: {_time.time()-t0:.1f}s", file=sys.stderr)
    t1 = _time.time()
    nc.finalize()
    print(f"[kernel] finalize: {_time.time()-t1:.1f}s", file=sys.stderr)
    return nc


def _get_nc(nstep: int, with_b2: bool):
    key = (nstep, with_b2) + _flags()
    if key not in _CACHE:
        _CACHE[key] = _build(nstep, with_b2)
    return _CACHE[key]


def _host_prep(coeffs, Wi1, bi1, Wi2, bi2, W1, b1, W2, b2, nstep: int):
    coeffs = np.asarray(coeffs, dtype=np.float32)
    a = coeffs[:, :, 0:8]
    b = coeffs[:, :, 8:16]
    c = coeffs[:, :, 16:24]
    d = coeffs[:, :, 24:32]

    X0 = a[:, 0]
    z0 = np.tanh(
        np.maximum(X0 @ Wi1 + bi1, 0.0).astype(np.float32) @ Wi2 + bi2
    ).astype(np.float32)

    # g with RK4 weights folded (cls1 column = 2x dXdt(t+1/2))
    g = np.empty((B, nstep, 3, C_IN), dtype=np.float32)
    g[:, :, 0] = b[:, :nstep]
    g[:, :, 1] = 2.0 * b[:, :nstep] + 2.0 * c[:, :nstep] + 1.5 * d[:, :nstep]
    last = NSTEP - 1
    for i in range(nstep):
        if i < last:
            g[:, i, 2] = b[:, i + 1]
        else:
            g[:, i, 2] = b[:, i] + 2.0 * c[:, i] + 3.0 * d[:, i]

    tcols = np.empty((nstep, 3), dtype=np.float32)
    tcols[:, 0] = np.arange(nstep, dtype=np.float32)
    tcols[:, 1] = tcols[:, 0] + 0.5
    tcols[:, 2] = tcols[:, 0] + 1.0
    bias1 = (
        b1[None, None, :] + tcols[:, :, None] * W1[0][None, None, :]
    ).astype(np.float32)
    bias1 = bias1.reshape(nstep * 3, MLP_H).T.copy()  # [128, nstep*3]

    # per-chunk selectors: s64[(hl*8+c), chunk*64 + h'] = 1 if h' == chunk*16+hl
    s64 = np.zeros((128, NCH * HID), dtype=np.float16)
    rows = np.arange(128)
    for cch in range(NCH):
        s64[rows, cch * HID + cch * HCH + rows // C_IN] = 1.0

    # W1S sets: alpha * (S_c @ W1z): row (hl, cin) = alpha * W1z[c*16+hl]
    w1z = W1[1:].astype(np.float32)  # [64, 128]
    w1s = np.empty((128, 3 * NCH * MLP_H), dtype=np.float16)
    for ai, alpha in enumerate((0.5, 0.25, 1.0 / 6.0)):
        for cch in range(NCH):
            blk = np.repeat(w1z[cch * HCH : (cch + 1) * HCH] * alpha,
                            C_IN, axis=0)  # [128, 128]
            w1s[:, (ai * NCH + cch) * MLP_H
                : (ai * NCH + cch + 1) * MLP_H] = blk

    zdt = np.float32  # f32r shares the f32 byte layout
    shared = {
        "bias1": bias1,
        "w1zr": np.ascontiguousarray(W1[1:], dtype=zdt),
        "w1zh": np.ascontiguousarray(W1[1:], dtype=np.float16),
        "w1s": w1s,
        "w2": np.ascontiguousarray(W2, dtype=np.float16),
        "s64": s64,
        "b2p": np.ascontiguousarray(b2[None, :], dtype=np.float32),
        "onesr": np.ones((1, BL), dtype=np.float32),
    }
    in_maps = []
    for core in range(NCORES):
        sl = slice(core * BL, (core + 1) * BL)
        mm = dict(shared)
        # gt[r, step, cls, b] = g[b, step, cls, r % 8], replicated 16x
        gcore = g[sl].transpose(3, 1, 2, 0)  # [8, nstep, 3, BL]
        gt = np.tile(gcore, (HCH, 1, 1, 1)).reshape(128, nstep * 3 * BL)
        mm["gt"] = np.ascontiguousarray(gt.astype(np.float16))
        z0t = np.ascontiguousarray(z0[sl].T)
        mm["z0t"] = z0t
        mm["m0"] = z0t.astype(np.float16)
        in_maps.append(mm)
    return in_maps


def kernel(coeffs, Wi1, bi1, Wi2, bi2, W1, b1, W2, b2, _nstep: int = NSTEP,
           _trace: bool = False):
    import sys
    import time as _time

    nstep = _nstep
    with_b2 = bool(np.any(np.asarray(b2)))
    nc = _get_nc(nstep, with_b2)
    in_maps = _host_prep(
        coeffs, Wi1, bi1, Wi2, bi2, W1, b1, W2, b2, nstep
    )
    t0 = _time.time()
    res = run_bass_kernel_spmd(nc, in_maps, list(range(NCORES)), trace=_trace)
    print(f"[kernel] spmd run (compile+exec): {_time.time()-t0:.1f}s", file=sys.stderr)
    out = np.empty((B, nstep + 1, HID), dtype=np.float32)
    for core in range(NCORES):
        zs = res.results[core]["zs"].reshape(HID, nstep + 1, BL)
        out[core * BL : (core + 1) * BL] = zs.transpose(2, 1, 0)
    if _trace:
        kernel.last_results = res
    return out
